# revision 1
# baseline (speedup 1.0000x reference)
"""Multi-head attention block (16 query heads, shared single K/V head) on
8 Trainium2 NeuronCores.

Reference computation (B=2, S=2048, D=2048, HQ=16, DH=128, fp32):
    q = (x @ Wq + bq)  -> [B, S, 16, 128]
    k = x @ Wk + bk    -> [B, S, 128]   (single shared K/V head)
    v = x @ Wv + bv    -> [B, S, 128]
    attn = softmax(q k^T / sqrt(128))
    out = (attn @ v) reshaped -> [B, S, D];  y = out @ Wo + bo

Sharding: batch x sequence-block data parallel. Core c handles batch c//4,
query rows (c%4)*512 .. +512, for ALL 16 heads. K/V projections are
replicated per core (they are cheap). This needs NO inter-core collectives
(an on-chip ReduceScatter at ~32 GB/s would cost more than the compute),
and every core emits a disjoint slab of the final output.

All matmuls run on the PE array in float32r (fp32 rounded to 11 mantissa
bits, streamed over 4 XBUSes at full PE rate for moving-dim >= 256, with
exact fp32 accumulation in PSUM; per-element rounding error ~1.2e-4).
Matmul operands are rounded host-side (weights, x) or produced as fp32r
on-device (qT/kT from DVE bias-add, p from ScalarE exp).

Softmax skips max-subtraction (scores are ~N(0,1) by construction; exp
cannot overflow), which lets scores live in the transposed [key, query]
layout end-to-end: exp on ScalarE straight PSUM->SBUF, then both p @ v and
the ones-row denominators contract the key axis (= partitions) on the PE
with no transposes. Denominators are divided into the head outputs before
the Wo projection.
"""

import numpy as np

B, S, D = 2, 2048, 2048
HQ, DH = 16, 128
SBLK = S // 4          # 512 query rows per core
N_CORES = 8
SCALE = 1.0 / float(np.sqrt(DH))

ND = D // 128          # 16 contraction chunks
NT = S // 128          # 16 key tiles
NQ = SBLK // 128       # 4 query row-tiles per core

_cache = {}


def _round_fp32r(a):
    """Round fp32 to fp32r (1s+8e+11m) with round-to-nearest-even-ish."""
    b = np.ascontiguousarray(a, dtype=np.float32).view(np.uint32)
    bias = np.uint32(0x7FF) + ((b >> np.uint32(12)) & np.uint32(1))
    return ((b + bias) & np.uint32(0xFFFFF000)).view(np.float32)


def _build():
    from concourse import bacc, mybir, tile
    from concourse.masks import make_identity

    F32 = mybir.dt.float32
    F32R = mybir.dt.float32r
    Exp = mybir.ActivationFunctionType.Exp
    mult = mybir.AluOpType.mult
    add = mybir.AluOpType.add

    nc = bacc.Bacc("TRN2", target_bir_lowering=False, debug=False,
                   num_devices=N_CORES)

    xT = nc.dram_tensor("xT", [D, S], F32R, kind="ExternalInput").ap()
    xTq = nc.dram_tensor("xTq", [D, SBLK], F32R, kind="ExternalInput").ap()
    Wq = nc.dram_tensor("Wq", [D, D], F32R, kind="ExternalInput").ap()
    bq = nc.dram_tensor("bq", [D], F32, kind="ExternalInput").ap()
    Wk = nc.dram_tensor("Wk", [D, DH], F32R, kind="ExternalInput").ap()
    bk = nc.dram_tensor("bk", [DH], F32, kind="ExternalInput").ap()
    Wv = nc.dram_tensor("Wv", [D, DH], F32R, kind="ExternalInput").ap()
    bv = nc.dram_tensor("bv", [DH], F32, kind="ExternalInput").ap()
    Wo = nc.dram_tensor("Wo", [D, D], F32R, kind="ExternalInput").ap()
    bo = nc.dram_tensor("bo", [D], F32R, kind="ExternalInput").ap()
    ones_d = nc.dram_tensor("ones", [128, 128], F32R, kind="ExternalInput").ap()
    y = nc.dram_tensor("y", [SBLK, D], F32, kind="ExternalOutput").ap()

    with tile.TileContext(nc) as tc, nc.allow_low_precision(
        reason="fp32r matmul pipeline; verified against fp32 reference"
    ):
        with (
            tc.tile_pool(name="const", bufs=1) as cpool,
            tc.tile_pool(name="live", bufs=1) as lpool,      # kT, v_nat, xq
            tc.tile_pool(name="ot", bufs=HQ) as otpool,      # 16 head outputs
        ):
            # ---- constants -------------------------------------------------
            ones = cpool.tile([128, 128], F32R)
            nc.sync.dma_start(out=ones[:, :], in_=ones_d[:, :])
            ones_col = ones[:, 0:1]
            ones_row = ones[0:1, :]
            ident = cpool.tile([128, 128], F32)
            make_identity(nc, ident[:, :])

            bk_col = cpool.tile([128, 1], F32)
            nc.sync.dma_start(out=bk_col[:, :], in_=bk[:].unsqueeze(1))
            bv_col = cpool.tile([128, 1], F32)
            nc.sync.dma_start(out=bv_col[:, :], in_=bv[:].unsqueeze(1))
            bq_cols = cpool.tile([128, HQ], F32)
            nc.sync.dma_start(
                out=bq_cols[:, :], in_=bq[:].rearrange("(h p) -> p h", p=128)
            )
            bo_row = cpool.tile([1, D], F32R)
            nc.sync.dma_start(out=bo_row[:, :], in_=bo[:].unsqueeze(0))

            kT = lpool.tile([128, S], F32R)
            v_nat = lpool.tile([128, NT, DH], F32R)
            xq = lpool.tile([128, ND, SBLK], F32R)
            nc.sync.dma_start(
                out=xq[:, :, :], in_=xTq.rearrange("(n p) s -> p n s", p=128)
            )

            # ---- phase A: k/v projections over the full sequence -----------
            # two half-sequence passes so the accumulators + transpose
            # staging fit in the 8 PSUM banks (2+2+2 = 6)
            with (
                tc.tile_pool(name="pha", bufs=1) as apool,
                tc.tile_pool(name="xa", bufs=4) as xpool,
                tc.tile_pool(name="pacc", bufs=1, space="PSUM") as pacc,
                tc.tile_pool(name="ptr", bufs=2, space="PSUM") as ptrp,
            ):
                wk_all = apool.tile([128, ND, DH], F32R)
                nc.sync.dma_start(
                    out=wk_all[:, :, :],
                    in_=Wk.rearrange("(n p) d -> p n d", p=128),
                )
                wv_all = apool.tile([128, ND, DH], F32R)
                nc.sync.dma_start(
                    out=wv_all[:, :, :],
                    in_=Wv.rearrange("(n p) d -> p n d", p=128),
                )
                vT = apool.tile([128, S], F32)

                HS = S // 2
                for th in range(2):
                    tsl = slice(th * HS, (th + 1) * HS)
                    psum_k = pacc.tile([128, HS], F32, tag="pk")
                    psum_v = pacc.tile([128, HS], F32, tag="pv")
                    for d in range(ND):
                        xt = xpool.tile([128, HS], F32R, tag="xt")
                        nc.sync.dma_start(
                            out=xt[:, :], in_=xT[d * 128:(d + 1) * 128, tsl]
                        )
                        for nb in range(HS // 512):
                            sl = slice(nb * 512, (nb + 1) * 512)
                            nc.tensor.matmul(
                                psum_k[:, sl],
                                lhsT=wk_all[:, d, :],
                                rhs=xt[:, sl],
                                start=(d == 0), stop=(d == ND - 1),
                            )
                            nc.tensor.matmul(
                                psum_v[:, sl],
                                lhsT=wv_all[:, d, :],
                                rhs=xt[:, sl],
                                start=(d == 0), stop=(d == ND - 1),
                            )

                    nc.vector.tensor_scalar(
                        kT[:, tsl], psum_k[:, :], bk_col[:, :], None, add
                    )
                    nc.vector.tensor_scalar(
                        vT[:, tsl], psum_v[:, :], bv_col[:, :], None, add
                    )

                # v in natural [key, DH] layout for the p@v contraction
                for t in range(NT):
                    ptr = ptrp.tile([128, 128], F32, tag="tr")
                    nc.tensor.transpose(
                        ptr[:, :], vT[:, t * 128:(t + 1) * 128], ident[:, :]
                    )
                    nc.vector.tensor_copy(v_nat[:, t, :], ptr[:, :])

            # ---- phase B: per-head q projection + attention ----------------
            outT_list = []
            with (
                tc.tile_pool(name="wq", bufs=3) as wqpool,
                tc.tile_pool(name="qt", bufs=2) as qtpool,
                tc.tile_pool(name="pt", bufs=3) as ptpool,
                tc.tile_pool(name="sm", bufs=2) as smpool,
                tc.tile_pool(name="psc", bufs=2, space="PSUM") as pscp,
                tc.tile_pool(name="pq", bufs=1, space="PSUM") as pqp,
                tc.tile_pool(name="po", bufs=1, space="PSUM") as pop,
                tc.tile_pool(name="pone", bufs=1, space="PSUM") as ponep,
            ):
                for h in range(HQ):
                    wq_t = wqpool.tile([128, ND, 128], F32R, tag="wq")
                    nc.sync.dma_start(
                        out=wq_t[:, :, :],
                        in_=Wq[:, h * 128:(h + 1) * 128].rearrange(
                            "(n p) m -> p n m", p=128
                        ),
                    )
                    pq = pqp.tile([128, SBLK], F32, tag="pq")
                    for d in range(ND):
                        nc.tensor.matmul(
                            pq[:, :],
                            lhsT=wq_t[:, d, :],
                            rhs=xq[:, d, :],
                            start=(d == 0), stop=(d == ND - 1),
                        )
                    qT = qtpool.tile([128, SBLK], F32R, tag="qt")
                    nc.vector.tensor_scalar(
                        qT[:, :], pq[:, :], bq_cols[:, h:h + 1], None, add
                    )

                    psum_o = pop.tile([128, SBLK], F32, tag="po")
                    psum_sum = ponep.tile([1, SBLK], F32, tag="psum")
                    for tp in range(NT // 2):
                        psc = pscp.tile([128, 2 * SBLK], F32, tag="sc")
                        for half in range(2):
                            t = tp * 2 + half
                            nc.tensor.matmul(
                                psc[:, half * SBLK:(half + 1) * SBLK],
                                lhsT=kT[:, t * 128:(t + 1) * 128],
                                rhs=qT[:, :],
                                start=True, stop=True,
                            )
                        pT = ptpool.tile([128, 2 * SBLK], F32R, tag="pT")
                        nc.scalar.activation(
                            pT[:, :], psc[:, :], Exp, scale=SCALE
                        )
                        for half in range(2):
                            t = tp * 2 + half
                            hs = slice(half * SBLK, (half + 1) * SBLK)
                            nc.tensor.matmul(
                                psum_o[:, :],
                                lhsT=v_nat[:, t, :],
                                rhs=pT[:, hs],
                                start=(t == 0), stop=(t == NT - 1),
                            )
                            nc.tensor.matmul(
                                psum_sum[:, :],
                                lhsT=ones_col,
                                rhs=pT[:, hs],
                                start=(t == 0), stop=(t == NT - 1),
                            )

                    recip = smpool.tile([1, SBLK], F32R, tag="recip")
                    nc.vector.reciprocal(recip[:, :], psum_sum[:, :])
                    pb = ponep.tile([128, SBLK], F32, tag="pb")
                    nc.tensor.matmul(
                        pb[:, :], lhsT=ones_row, rhs=recip[:, :],
                        start=True, stop=True,
                    )
                    recip_b = smpool.tile([128, SBLK], F32, tag="recipb")
                    nc.scalar.copy(recip_b[:, :], pb[:, :])
                    outT = otpool.tile([128, SBLK], F32R, tag="outT")
                    nc.vector.tensor_tensor(
                        outT[:, :], psum_o[:, :], recip_b[:, :], mult
                    )
                    outT_list.append(outT)

            # ---- phase C: output projection y = out @ Wo + bo --------------
            with (
                tc.tile_pool(name="wo", bufs=36) as wopool,
                tc.tile_pool(name="yp", bufs=3) as ypool,
                tc.tile_pool(name="bo", bufs=1) as bopool,
                tc.tile_pool(name="py", bufs=2, space="PSUM") as pyp,
                tc.tile_pool(name="pbo", bufs=2, space="PSUM") as pbop,
            ):
                bo_b = bopool.tile([128, D], F32)
                for nb in range(D // 512):
                    pbo = pbop.tile([128, 512], F32, tag="bo")
                    nc.tensor.matmul(
                        pbo[:, :],
                        lhsT=ones_row,
                        rhs=bo_row[:, nb * 512:(nb + 1) * 512],
                        start=True, stop=True,
                    )
                    nc.scalar.copy(bo_b[:, nb * 512:(nb + 1) * 512], pbo[:, :])

                for db in range(D // 512):
                    dsl = slice(db * 512, (db + 1) * 512)
                    wo_tiles = []
                    for hh in range(HQ):
                        wt = wopool.tile([128, 512], F32R, tag="wo")
                        nc.sync.dma_start(
                            out=wt[:, :], in_=Wo[hh * 128:(hh + 1) * 128, dsl]
                        )
                        wo_tiles.append(wt)
                    for st in range(NQ):
                        py = pyp.tile([128, 512], F32, tag="py")
                        for hh in range(HQ):
                            nc.tensor.matmul(
                                py[:, :],
                                lhsT=outT_list[hh][:, st * 128:(st + 1) * 128],
                                rhs=wo_tiles[hh][:, :],
                                start=(hh == 0), stop=(hh == HQ - 1),
                            )
                        y_sb = ypool.tile([128, 512], F32, tag="y")
                        nc.vector.tensor_tensor(
                            y_sb[:, :], py[:, :], bo_b[:, dsl], add
                        )
                        nc.sync.dma_start(
                            out=y[st * 128:(st + 1) * 128, dsl], in_=y_sb[:, :]
                        )

    nc.compile()
    return nc


def _get_nc():
    if "nc" not in _cache:
        _cache["nc"] = _build()
    return _cache["nc"]


def _prepare_in_maps(x, Wq, bq, Wk, bk, Wv, bv, Wo, bo):
    x = np.asarray(x, dtype=np.float32)
    bq = np.asarray(bq, dtype=np.float32)
    bk = np.asarray(bk, dtype=np.float32)
    bv = np.asarray(bv, dtype=np.float32)
    Wq = _round_fp32r(Wq)
    Wk = _round_fp32r(Wk)
    Wv = _round_fp32r(Wv)
    Wo = _round_fp32r(Wo)
    bo = _round_fp32r(bo)
    ones = np.ones((128, 128), np.float32)

    xT = [_round_fp32r(np.asarray(x[g], np.float32).T) for g in range(B)]
    in_maps = []
    for c in range(N_CORES):
        g, blk = divmod(c, 4)
        s0 = blk * SBLK
        in_maps.append({
            "xT": xT[g],
            "xTq": np.ascontiguousarray(xT[g][:, s0:s0 + SBLK]),
            "Wq": Wq, "bq": bq, "Wk": Wk, "bk": bk,
            "Wv": Wv, "bv": bv, "Wo": Wo, "bo": bo, "ones": ones,
        })
    return in_maps


def _assemble(results):
    out = np.empty((B, S, D), dtype=np.float32)
    for c in range(N_CORES):
        g, blk = divmod(c, 4)
        out[g, blk * SBLK:(blk + 1) * SBLK, :] = results[c]["y"]
    return out


def kernel(x, Wq, bq, Wk, bk, Wv, bv, Wo, bo):
    from concourse.bass_utils import run_bass_kernel_spmd

    in_maps = _prepare_in_maps(x, Wq, bq, Wk, bk, Wv, bv, Wo, bo)
    nc = _get_nc()
    res = run_bass_kernel_spmd(nc, in_maps, core_ids=list(range(N_CORES)))
    return _assemble(res.results)



# revision 18
# speedup vs baseline: 1.2377x; 1.2377x over previous
"""Multi-head attention block (16 query heads, shared single K/V head) on
8 Trainium2 NeuronCores — fp16 pipeline.

Reference computation (B=2, S=2048, D=2048, HQ=16, DH=128, fp32):
    q = (x @ Wq + bq)  -> [B, S, 16, 128]
    k = x @ Wk + bk    -> [B, S, 128]   (single shared K/V head)
    v = x @ Wv + bv    -> [B, S, 128]
    attn = softmax(q k^T / sqrt(128))
    out = (attn @ v) reshaped -> [B, S, D];  y = out @ Wo + bo
    (dropout is identity in eval)

Sharding: batch x sequence-block data parallel (no collectives). Core c
handles batch c//4, query rows (c%4)*512 .. +512, for ALL 16 heads; K/V
over the full sequence are recomputed per core (cheap).

Precision: the attention output is a softmax-weighted mean over ~750
effective keys, so its magnitude is ~27x smaller than v's; quantization
noise on any matmul operand passes through to the output at roughly its
per-element RMS. fp8's ~4%/element is far too coarse, so every operand
runs fp16 (~0.1%/element, full PE rate, fast-weight-load eligible) with
fp32 PSUM accumulation. Measured end-to-end error vs the fp32 reference
is ~2e-3.

Structure: scores stay in the transposed [key, query] layout end-to-end
(softmax skips max-subtraction; scores ~N(0,1) by construction, and exp
applies a constant -3 offset that cancels in the normalization). exp runs
on ScalarE straight PSUM->SBUF; p@v contracts the key axis on the PE with
no transposes. Softmax denominators come from DVE tile-accumulation of p
plus one tiny ones-matmul per head (saving ~120k PE cycles vs per-tile
ones-matmuls); the per-head normalization uses reciprocal_approx_fast and
a PE row-broadcast. The Wo projection runs in 4 rounds of 4 heads so 3/4
of it overlaps the attention loop.
"""

import numpy as np

B, S, D = 2, 2048, 2048
HQ, DH = 16, 128
SBLK = S // 4          # 512 query rows per core
N_CORES = 8
SCALE = 1.0 / float(np.sqrt(DH))
EXP_BIAS = -3.0        # exp(s/sqrt(DH) - 3): cancels in softmax

ND = D // 128          # 16 contraction chunks
NT = S // 128          # 16 key tiles
NQ = SBLK // 128       # 4 query row-tiles per core

_cache = {}


def _round_fp32r(a):
    """Round fp32 to fp32r (1s+8e+11m) with round-to-nearest-even-ish."""
    b = np.ascontiguousarray(a, dtype=np.float32).view(np.uint32)
    bias = np.uint32(0x7FF) + ((b >> np.uint32(12)) & np.uint32(1))
    return ((b + bias) & np.uint32(0xFFFFF000)).view(np.float32)


def _to_f16(a):
    return np.ascontiguousarray(np.asarray(a, np.float32)).astype(np.float16)


def _build():
    from concourse import bacc, mybir, tile
    from concourse.masks import make_identity

    F32 = mybir.dt.float32
    F32R = mybir.dt.float32r
    F16 = mybir.dt.float16
    Exp = mybir.ActivationFunctionType.Exp
    mult = mybir.AluOpType.mult
    add = mybir.AluOpType.add

    nc = bacc.Bacc("TRN2", target_bir_lowering=False, debug=False,
                   num_devices=N_CORES)

    # pre-rearranged on host; see _prepare_in_maps
    xall_d = nc.dram_tensor("xall", [128, ND, S], F16, kind="ExternalInput").ap()
    xq_d = nc.dram_tensor("xq", [128, ND, SBLK], F16, kind="ExternalInput").ap()
    Wq = nc.dram_tensor("Wq", [128, HQ, ND, 128], F16, kind="ExternalInput").ap()
    bq = nc.dram_tensor("bq", [D], F32, kind="ExternalInput").ap()
    Wk = nc.dram_tensor("Wk", [128, ND, DH], F16, kind="ExternalInput").ap()
    bk = nc.dram_tensor("bk", [DH], F32, kind="ExternalInput").ap()
    Wv = nc.dram_tensor("Wv", [128, ND, DH], F16, kind="ExternalInput").ap()
    bv = nc.dram_tensor("bv", [DH], F32, kind="ExternalInput").ap()
    Wo = nc.dram_tensor("Wo", [128, HQ, 4, 512], F16, kind="ExternalInput").ap()
    bo = nc.dram_tensor("bo", [1, D], F32R, kind="ExternalInput").ap()
    ones16_d = nc.dram_tensor("ones16", [128, 16], F16, kind="ExternalInput").ap()
    onesr_d = nc.dram_tensor("onesr", [1, 128], F32R, kind="ExternalInput").ap()
    y = nc.dram_tensor("y", [SBLK, D], F32, kind="ExternalOutput").ap()

    with tile.TileContext(nc) as tc, nc.allow_low_precision(
        reason="fp16 matmul pipeline; verified against fp32 reference"
    ):
        with (
            tc.tile_pool(name="const", bufs=1) as cpool,
            tc.tile_pool(name="live", bufs=1) as lpool,
            tc.tile_pool(name="ot", bufs=HQ // 2) as otpool,  # 8 head-pair outs
            tc.tile_pool(name="ya", bufs=16) as yapool,       # y accumulators
            tc.tile_pool(name="wo", bufs=20) as wopool,
            tc.tile_pool(name="yw", bufs=3) as ypool,
            tc.tile_pool(name="rc", bufs=2) as rcpool,
        ):
            # ---- constants -------------------------------------------------
            ones16 = cpool.tile([128, 16], F16)
            nc.sync.dma_start(out=ones16[:, :], in_=ones16_d[:, :])
            ones_col = ones16[:, 0:1]
            ones_row = cpool.tile([1, 128], F32R)
            nc.sync.dma_start(out=ones_row[:, :], in_=onesr_d[:, :])
            ident = cpool.tile([128, 128], F32)
            make_identity(nc, ident[:, :])
            ebias_col = cpool.tile([128, 1], F32)
            nc.gpsimd.memset(ebias_col[:, :], EXP_BIAS)

            bk_col = cpool.tile([128, 1], F32)
            nc.sync.dma_start(out=bk_col[:, :], in_=bk[:].unsqueeze(1))
            bv_col = cpool.tile([128, 1], F32)
            nc.sync.dma_start(out=bv_col[:, :], in_=bv[:].unsqueeze(1))
            bq_cols = cpool.tile([128, HQ], F32)
            nc.sync.dma_start(
                out=bq_cols[:, :], in_=bq[:].rearrange("(h p) -> p h", p=128)
            )
            bo_row = cpool.tile([1, D], F32R)
            nc.sync.dma_start(out=bo_row[:, :], in_=bo[:, :])

            xq = lpool.tile([128, ND, SBLK], F16)
            nc.sync.dma_start(out=xq[:, :, :], in_=xq_d[:, :, :])
            kT = lpool.tile([128, S], F16)
            v_nat = lpool.tile([128, NT, DH], F16)

            # ---- phase A: k/v projections over the full sequence -----------
            # xall is scoped here so its 64KB/partition frees before phase B.
            with (
                tc.tile_pool(name="pha", bufs=1) as apool,
                tc.tile_pool(name="pacc", bufs=1, space="PSUM") as pacc,
                tc.tile_pool(name="ptr", bufs=2, space="PSUM") as ptrp,
            ):
                wk_all = apool.tile([128, ND, DH], F16)
                nc.sync.dma_start(out=wk_all[:, :, :], in_=Wk[:, :, :])
                wv_all = apool.tile([128, ND, DH], F16)
                nc.sync.dma_start(out=wv_all[:, :, :], in_=Wv[:, :, :])
                xall = apool.tile([128, ND, S], F16)
                for c4 in range(4):
                    nc.sync.dma_start(
                        out=xall[:, c4 * 4:(c4 + 1) * 4, :],
                        in_=xall_d[:, c4 * 4:(c4 + 1) * 4, :],
                    )
                vT = apool.tile([128, S], F32)

                HS = S // 2
                for th in range(2):
                    tsl = slice(th * HS, (th + 1) * HS)
                    psum_k = pacc.tile([128, HS], F32, tag="pk")
                    psum_v = pacc.tile([128, HS], F32, tag="pv")
                    for d in range(ND):
                        for nb in range(HS // 512):
                            sl = slice(nb * 512, (nb + 1) * 512)
                            xsl = slice(th * HS + nb * 512,
                                        th * HS + (nb + 1) * 512)
                            nc.tensor.matmul(
                                psum_k[:, sl],
                                lhsT=wk_all[:, d, :],
                                rhs=xall[:, d, xsl],
                                start=(d == 0), stop=(d == ND - 1),
                            )
                            nc.tensor.matmul(
                                psum_v[:, sl],
                                lhsT=wv_all[:, d, :],
                                rhs=xall[:, d, xsl],
                                start=(d == 0), stop=(d == ND - 1),
                            )

                    nc.vector.tensor_scalar(
                        kT[:, tsl], psum_k[:, :], bk_col[:, :], None, add
                    )
                    nc.vector.tensor_scalar(
                        vT[:, tsl], psum_v[:, :], bv_col[:, :], None, add
                    )

                # v in natural [key, DH] layout for the p@v contraction
                for t in range(NT):
                    ptr = ptrp.tile([128, 128], F32, tag="tr")
                    nc.tensor.transpose(
                        ptr[:, :], vT[:, t * 128:(t + 1) * 128], ident[:, :]
                    )
                    nc.vector.tensor_copy(v_nat[:, t, :], ptr[:, :])

            # ---- phase B (attention) + phase C (Wo) interleaved ------------
            outT_pairs = []
            yacc_tiles = {}
            wo_tiles = {}

            with (
                tc.tile_pool(name="wq", bufs=3) as wqpool,
                tc.tile_pool(name="qt", bufs=2) as qtpool,
                tc.tile_pool(name="pt", bufs=3) as ptpool,
                tc.tile_pool(name="dac", bufs=2) as dacpool,
                tc.tile_pool(name="ps", bufs=2, space="PSUM") as pspool,
                tc.tile_pool(name="po", bufs=1, space="PSUM") as popool,
                tc.tile_pool(name="aux", bufs=2, space="PSUM") as auxpool,
                tc.tile_pool(name="py", bufs=1, space="PSUM") as pypool,
            ):
                # bo broadcast [1,D] -> [128,D] via PE
                bo_b = cpool.tile([128, D], F32)
                for nb in range(D // 512):
                    pbo = pypool.tile([128, 512], F32, tag="py")
                    nc.tensor.matmul(
                        pbo[:, :], lhsT=ones_row[0:1, :],
                        rhs=bo_row[:, nb * 512:(nb + 1) * 512],
                        start=True, stop=True,
                    )
                    nc.scalar.copy(bo_b[:, nb * 512:(nb + 1) * 512], pbo[:, :])

                def wo_round(r):
                    """Accumulate heads 4r..4r+3 into the y accumulators."""
                    for st in range(NQ):
                        for db in range(4):
                            py = pypool.tile([128, 512], F32, tag="py")
                            for j in range(4):
                                h = 4 * r + j
                                pp, hi = divmod(h, 2)
                                nc.tensor.matmul(
                                    py[:, :],
                                    lhsT=outT_pairs[pp][
                                        :, hi, st * 128:(st + 1) * 128],
                                    rhs=wo_tiles[(h, db)][:, :],
                                    start=(j == 0), stop=(j == 3),
                                )
                            if r == 0:
                                ya = yapool.tile([128, 512], F32, tag="ya")
                                yacc_tiles[(st, db)] = ya
                                nc.vector.tensor_tensor(
                                    ya[:, :], py[:, :],
                                    bo_b[:, db * 512:(db + 1) * 512], add,
                                )
                            elif r < 3:
                                ya = yacc_tiles[(st, db)]
                                nc.vector.tensor_tensor(
                                    ya[:, :], py[:, :], ya[:, :], add,
                                )
                            else:
                                ya = yacc_tiles[(st, db)]
                                y_sb = ypool.tile([128, 512], F32, tag="y")
                                nc.vector.tensor_tensor(
                                    y_sb[:, :], py[:, :], ya[:, :], add,
                                )
                                nc.sync.dma_start(
                                    out=y[st * 128:(st + 1) * 128,
                                          db * 512:(db + 1) * 512],
                                    in_=y_sb[:, :],
                                )

                def wo_fetch(h4):
                    for hh in range(h4, h4 + 4):
                        for db in range(4):
                            wt = wopool.tile([128, 512], F16, tag="wo")
                            nc.sync.dma_start(
                                out=wt[:, :], in_=Wo[:, hh, db, :]
                            )
                            wo_tiles[(hh, db)] = wt

                wo_fetch(0)

                for h in range(HQ):
                    pp, hi = divmod(h, 2)
                    wq_t = wqpool.tile([128, ND, 128], F16, tag="wq")
                    nc.sync.dma_start(out=wq_t[:, :, :], in_=Wq[:, h, :, :])

                    pq = pspool.tile([128, 2, SBLK], F32, tag="sc")
                    for d in range(ND):
                        nc.tensor.matmul(
                            pq[:, 0, :],
                            lhsT=wq_t[:, d, :],
                            rhs=xq[:, d, :],
                            start=(d == 0), stop=(d == ND - 1),
                        )
                    qT = qtpool.tile([128, SBLK], F16, tag="qt")
                    nc.vector.tensor_scalar(
                        qT[:, :], pq[:, 0, :], bq_cols[:, h:h + 1], None, add
                    )

                    if hi == 0:
                        outT = otpool.tile([128, 2, SBLK], F16, tag="ot")
                        outT_pairs.append(outT)
                    outT = outT_pairs[pp]

                    psum_o = popool.tile([128, SBLK], F32, tag="po")
                    dacc = dacpool.tile([128, 2, SBLK], F16, tag="da")
                    for tp in range(NT // 2):
                        psc = pspool.tile([128, 2, SBLK], F32, tag="sc")
                        for half in range(2):
                            t = tp * 2 + half
                            nc.tensor.matmul(
                                psc[:, half, :],
                                lhsT=kT[:, t * 128:(t + 1) * 128],
                                rhs=qT[:, :],
                                start=True, stop=True,
                            )
                        pT = ptpool.tile([128, 2, SBLK], F16, tag="pT")
                        nc.scalar.activation(
                            pT[:, :, :], psc[:, :, :], Exp,
                            bias=ebias_col[:, :], scale=SCALE,
                        )
                        for half in range(2):
                            t = tp * 2 + half
                            nc.tensor.matmul(
                                psum_o[:, :],
                                lhsT=v_nat[:, t, :],
                                rhs=pT[:, half, :],
                                start=(t == 0), stop=(t == NT - 1),
                            )
                        # denominator: elementwise-accumulate p on DVE
                        if tp == 0:
                            nc.vector.tensor_copy(dacc[:, :, :], pT[:, :, :])
                        else:
                            nc.vector.tensor_tensor(
                                dacc[:, :, :], dacc[:, :, :], pT[:, :, :], add
                            )

                    psum_den = auxpool.tile([128, SBLK], F32, tag="aux")
                    for half in range(2):
                        nc.tensor.matmul(
                            psum_den[0:1, :],
                            lhsT=ones_col,
                            rhs=dacc[:, half, :],
                            start=(half == 0), stop=(half == 1),
                        )
                    recip_f = rcpool.tile([1, SBLK], F32, tag="rcf")
                    nc.vector.reciprocal_approx_fast(
                        recip_f[0:1, :], psum_den[0:1, :]
                    )
                    recip = rcpool.tile([1, SBLK], F32R, tag="rc")
                    nc.vector.tensor_copy(recip[0:1, :], recip_f[0:1, :])
                    pb = auxpool.tile([128, SBLK], F32, tag="aux")
                    nc.tensor.matmul(
                        pb[:, :], lhsT=ones_row[0:1, :],
                        rhs=recip[0:1, :],
                        start=True, stop=True,
                    )
                    recip_b = rcpool.tile([128, SBLK], F32, tag="rb")
                    nc.scalar.copy(recip_b[:, :], pb[:, :])
                    nc.vector.tensor_tensor(
                        outT[:, hi, :], psum_o[:, :], recip_b[:, :], mult
                    )

                    # overlap Wo DMA + rounds with the attention loop
                    if h == 3:
                        wo_round(0)
                        wo_fetch(4)
                    elif h == 7:
                        wo_round(1)
                        wo_fetch(8)
                    elif h == 11:
                        wo_round(2)
                        wo_fetch(12)

                wo_round(3)

    nc.compile()
    return nc


def _get_nc():
    if "nc" not in _cache:
        _cache["nc"] = _build()
    return _cache["nc"]


def _prepare_in_maps(x, Wq, bq, Wk, bk, Wv, bv, Wo, bo):
    x = np.asarray(x, dtype=np.float32)
    bq = np.asarray(bq, dtype=np.float32)
    bk = np.asarray(bk, dtype=np.float32)
    bv = np.asarray(bv, dtype=np.float32)

    Wq = np.asarray(Wq, np.float32)
    Wk = np.asarray(Wk, np.float32)
    Wv = np.asarray(Wv, np.float32)
    Wo = np.asarray(Wo, np.float32)

    # [p, h, n, m] <- Wq[n*128+p, h*128+m]  (contiguous 2KB DMA lines)
    Wq_pre = _to_f16(Wq.reshape(ND, 128, HQ, 128).transpose(1, 2, 0, 3))
    Wk_pre = _to_f16(Wk.reshape(ND, 128, DH).transpose(1, 0, 2))
    Wv_pre = _to_f16(Wv.reshape(ND, 128, DH).transpose(1, 0, 2))
    # [p, h, db, m] <- Wo[h*128+p, db*512+m]
    Wo_pre = _to_f16(Wo.reshape(HQ, 128, 4, 512).transpose(1, 0, 2, 3))
    bo_pre = _round_fp32r(np.asarray(bo, np.float32)).reshape(1, D)

    ones16 = np.ones((128, 16), np.float16)
    onesr = np.ones((1, 128), np.float32)

    # xT_pre[g]: [p, n, s] <- x[g].T[n*128+p, s]
    xT_pre = [
        _to_f16(x[g].T.reshape(ND, 128, S).transpose(1, 0, 2))
        for g in range(B)
    ]
    in_maps = []
    for c in range(N_CORES):
        g, blk = divmod(c, 4)
        s0 = blk * SBLK
        in_maps.append({
            "xall": xT_pre[g],
            "xq": np.ascontiguousarray(xT_pre[g][:, :, s0:s0 + SBLK]),
            "Wq": Wq_pre, "bq": bq, "Wk": Wk_pre, "bk": bk,
            "Wv": Wv_pre, "bv": bv, "Wo": Wo_pre, "bo": bo_pre,
            "ones16": ones16, "onesr": onesr,
        })
    return in_maps


def _assemble(results):
    out = np.empty((B, S, D), dtype=np.float32)
    for c in range(N_CORES):
        g, blk = divmod(c, 4)
        out[g, blk * SBLK:(blk + 1) * SBLK, :] = results[c]["y"]
    return out


def kernel(x, Wq, bq, Wk, bk, Wv, bv, Wo, bo):
    from concourse.bass_utils import run_bass_kernel_spmd

    in_maps = _prepare_in_maps(x, Wq, bq, Wk, bk, Wv, bv, Wo, bo)
    nc = _get_nc()
    res = run_bass_kernel_spmd(nc, in_maps, core_ids=list(range(N_CORES)))
    return _assemble(res.results)


# revision 20
# speedup vs baseline: 1.3229x; 1.0688x over previous
"""Multi-head attention block (16 query heads, shared single K/V head) on
8 Trainium2 NeuronCores — fp16 pipeline.

Reference computation (B=2, S=2048, D=2048, HQ=16, DH=128, fp32):
    q = (x @ Wq + bq)  -> [B, S, 16, 128]
    k = x @ Wk + bk    -> [B, S, 128]   (single shared K/V head)
    v = x @ Wv + bv    -> [B, S, 128]
    attn = softmax(q k^T / sqrt(128))
    out = (attn @ v) reshaped -> [B, S, D];  y = out @ Wo + bo
    (dropout is identity in eval)

Sharding: batch x sequence-block data parallel (no collectives). Core c
handles batch c//4, query rows (c%4)*512 .. +512, for ALL 16 heads; K/V
over the full sequence are recomputed per core (cheap).

Precision: the attention output is a softmax-weighted mean over ~750
effective keys, so its magnitude is ~27x smaller than v's; quantization
noise on any matmul operand passes through to the output at roughly its
per-element RMS. fp8's ~4%/element is far too coarse, so every operand
runs fp16 (~0.1%/element, full PE rate, fast-weight-load eligible) with
fp32 PSUM accumulation. Measured end-to-end error vs the fp32 reference
is ~2e-3.

Structure: scores stay in the transposed [key, query] layout end-to-end
(softmax skips max-subtraction; scores ~N(0,1) by construction, and exp
applies a constant -3 offset that cancels in the normalization). exp runs
on ScalarE straight PSUM->SBUF; p@v contracts the key axis on the PE with
no transposes. Softmax denominators come from DVE tile-accumulation of p
plus one tiny ones-matmul per head (saving ~120k PE cycles vs per-tile
ones-matmuls); the per-head normalization uses reciprocal_approx_fast and
a PE row-broadcast. The Wo projection runs in 4 rounds of 4 heads so 3/4
of it overlaps the attention loop.
"""

import numpy as np

B, S, D = 2, 2048, 2048
HQ, DH = 16, 128
SBLK = S // 4          # 512 query rows per core
N_CORES = 8
SCALE = 1.0 / float(np.sqrt(DH))
EXP_BIAS = -3.0        # exp(s/sqrt(DH) - 3): cancels in softmax

ND = D // 128          # 16 contraction chunks
NT = S // 128          # 16 key tiles
NQ = SBLK // 128       # 4 query row-tiles per core

_cache = {}


def _round_fp32r(a):
    """Round fp32 to fp32r (1s+8e+11m) with round-to-nearest-even-ish."""
    b = np.ascontiguousarray(a, dtype=np.float32).view(np.uint32)
    bias = np.uint32(0x7FF) + ((b >> np.uint32(12)) & np.uint32(1))
    return ((b + bias) & np.uint32(0xFFFFF000)).view(np.float32)


def _to_f16(a):
    return np.ascontiguousarray(np.asarray(a, np.float32)).astype(np.float16)


def _build():
    from concourse import bacc, mybir, tile
    from concourse.masks import make_identity

    F32 = mybir.dt.float32
    F32R = mybir.dt.float32r
    F16 = mybir.dt.float16
    Exp = mybir.ActivationFunctionType.Exp
    mult = mybir.AluOpType.mult
    add = mybir.AluOpType.add

    nc = bacc.Bacc("TRN2", target_bir_lowering=False, debug=False,
                   num_devices=N_CORES)

    # pre-rearranged on host; see _prepare_in_maps
    xall_d = nc.dram_tensor("xall", [128, ND, S], F16, kind="ExternalInput").ap()
    xq_d = nc.dram_tensor("xq", [128, ND, SBLK], F16, kind="ExternalInput").ap()
    Wq = nc.dram_tensor("Wq", [128, HQ, ND, 128], F16, kind="ExternalInput").ap()
    bq = nc.dram_tensor("bq", [D], F32, kind="ExternalInput").ap()
    Wk = nc.dram_tensor("Wk", [128, ND, DH], F16, kind="ExternalInput").ap()
    bk = nc.dram_tensor("bk", [DH], F32, kind="ExternalInput").ap()
    Wv = nc.dram_tensor("Wv", [128, ND, DH], F16, kind="ExternalInput").ap()
    bv = nc.dram_tensor("bv", [DH], F32, kind="ExternalInput").ap()
    Wo = nc.dram_tensor("Wo", [128, HQ, 4, 512], F16, kind="ExternalInput").ap()
    bo = nc.dram_tensor("bo", [1, D], F32R, kind="ExternalInput").ap()
    ones16_d = nc.dram_tensor("ones16", [128, 16], F16, kind="ExternalInput").ap()
    onesr_d = nc.dram_tensor("onesr", [1, 128], F32R, kind="ExternalInput").ap()
    y = nc.dram_tensor("y", [SBLK, D], F32, kind="ExternalOutput").ap()

    with tile.TileContext(nc) as tc, nc.allow_low_precision(
        reason="fp16 matmul pipeline; verified against fp32 reference"
    ):
        with (
            tc.tile_pool(name="const", bufs=1) as cpool,
            tc.tile_pool(name="live", bufs=1) as lpool,
            tc.tile_pool(name="ot", bufs=HQ // 2) as otpool,  # 8 head-pair outs
            tc.tile_pool(name="ya", bufs=16) as yapool,       # y accumulators
            tc.tile_pool(name="wo", bufs=20) as wopool,
            tc.tile_pool(name="yw", bufs=3) as ypool,
            tc.tile_pool(name="rc", bufs=2) as rcpool,
        ):
            # ---- constants -------------------------------------------------
            ones16 = cpool.tile([128, 16], F16)
            nc.sync.dma_start(out=ones16[:, :], in_=ones16_d[:, :])
            ones_col = ones16[:, 0:1]
            ones_row = cpool.tile([1, 128], F32R)
            nc.sync.dma_start(out=ones_row[:, :], in_=onesr_d[:, :])
            ident = cpool.tile([128, 128], F32)
            make_identity(nc, ident[:, :])
            ebias_col = cpool.tile([128, 1], F32)
            nc.gpsimd.memset(ebias_col[:, :], EXP_BIAS)

            bk_col = cpool.tile([128, 1], F32)
            nc.sync.dma_start(out=bk_col[:, :], in_=bk[:].unsqueeze(1))
            bv_col = cpool.tile([128, 1], F32)
            nc.sync.dma_start(out=bv_col[:, :], in_=bv[:].unsqueeze(1))
            bq_cols = cpool.tile([128, HQ], F32)
            nc.sync.dma_start(
                out=bq_cols[:, :], in_=bq[:].rearrange("(h p) -> p h", p=128)
            )
            bo_row = cpool.tile([1, D], F32R)
            nc.sync.dma_start(out=bo_row[:, :], in_=bo[:, :])

            xq = lpool.tile([128, ND, SBLK], F16)
            kT = lpool.tile([128, S], F16)
            v_nat = lpool.tile([128, NT, DH], F16)

            # ---- phase A: k/v projections over the full sequence -----------
            # xall is scoped here so its 64KB/partition frees before phase B.
            with (
                tc.tile_pool(name="pha", bufs=1) as apool,
                tc.tile_pool(name="pacc", bufs=1, space="PSUM") as pacc,
                tc.tile_pool(name="ptr", bufs=2, space="PSUM") as ptrp,
            ):
                wk_all = apool.tile([128, ND, DH], F16)
                nc.sync.dma_start(out=wk_all[:, :, :], in_=Wk[:, :, :])
                wv_all = apool.tile([128, ND, DH], F16)
                nc.sync.dma_start(out=wv_all[:, :, :], in_=Wv[:, :, :])
                xall = apool.tile([128, ND, S], F16)
                for c8 in range(8):
                    nc.sync.dma_start(
                        out=xall[:, c8 * 2:(c8 + 1) * 2, :],
                        in_=xall_d[:, c8 * 2:(c8 + 1) * 2, :],
                    )
                    if c8 == 0:
                        nc.sync.dma_start(out=xq[:, :, :], in_=xq_d[:, :, :])
                vT = apool.tile([128, S], F32)

                HS = S // 2
                for th in range(2):
                    tsl = slice(th * HS, (th + 1) * HS)
                    psum_k = pacc.tile([128, HS], F32, tag="pk")
                    psum_v = pacc.tile([128, HS], F32, tag="pv")
                    for d in range(ND):
                        for nb in range(HS // 512):
                            sl = slice(nb * 512, (nb + 1) * 512)
                            xsl = slice(th * HS + nb * 512,
                                        th * HS + (nb + 1) * 512)
                            nc.tensor.matmul(
                                psum_k[:, sl],
                                lhsT=wk_all[:, d, :],
                                rhs=xall[:, d, xsl],
                                start=(d == 0), stop=(d == ND - 1),
                            )
                            nc.tensor.matmul(
                                psum_v[:, sl],
                                lhsT=wv_all[:, d, :],
                                rhs=xall[:, d, xsl],
                                start=(d == 0), stop=(d == ND - 1),
                            )

                    nc.vector.tensor_scalar(
                        kT[:, tsl], psum_k[:, :], bk_col[:, :], None, add
                    )
                    nc.vector.tensor_scalar(
                        vT[:, tsl], psum_v[:, :], bv_col[:, :], None, add
                    )

                # v in natural [key, DH] layout for the p@v contraction
                for t in range(NT):
                    ptr = ptrp.tile([128, 128], F32, tag="tr")
                    nc.tensor.transpose(
                        ptr[:, :], vT[:, t * 128:(t + 1) * 128], ident[:, :]
                    )
                    nc.vector.tensor_copy(v_nat[:, t, :], ptr[:, :])

            # ---- phase B (attention) + phase C (Wo) interleaved ------------
            outT_pairs = []
            yacc_tiles = {}
            wo_tiles = {}

            with (
                tc.tile_pool(name="wq", bufs=3) as wqpool,
                tc.tile_pool(name="qt", bufs=2) as qtpool,
                tc.tile_pool(name="pt", bufs=3) as ptpool,
                tc.tile_pool(name="dac", bufs=2) as dacpool,
                tc.tile_pool(name="ps", bufs=2, space="PSUM") as pspool,
                tc.tile_pool(name="po", bufs=2, space="PSUM") as popool,
                tc.tile_pool(name="aux", bufs=1, space="PSUM") as auxpool,
                tc.tile_pool(name="py", bufs=1, space="PSUM") as pypool,
            ):
                # bo broadcast [1,D] -> [128,D] via PE
                bo_b = cpool.tile([128, D], F32)
                for nb in range(D // 512):
                    pbo = pypool.tile([128, 512], F32, tag="py")
                    nc.tensor.matmul(
                        pbo[:, :], lhsT=ones_row[0:1, :],
                        rhs=bo_row[:, nb * 512:(nb + 1) * 512],
                        start=True, stop=True,
                    )
                    nc.scalar.copy(bo_b[:, nb * 512:(nb + 1) * 512], pbo[:, :])

                def wo_round(r):
                    """Accumulate heads 4r..4r+3 into the y accumulators."""
                    for st in range(NQ):
                        for db in range(4):
                            py = pypool.tile([128, 512], F32, tag="py")
                            for j in range(4):
                                h = 4 * r + j
                                pp, hi = divmod(h, 2)
                                nc.tensor.matmul(
                                    py[:, :],
                                    lhsT=outT_pairs[pp][
                                        :, hi, st * 128:(st + 1) * 128],
                                    rhs=wo_tiles[(h, db)][:, :],
                                    start=(j == 0), stop=(j == 3),
                                )
                            if r == 0:
                                ya = yapool.tile([128, 512], F32, tag="ya")
                                yacc_tiles[(st, db)] = ya
                                nc.vector.tensor_tensor(
                                    ya[:, :], py[:, :],
                                    bo_b[:, db * 512:(db + 1) * 512], add,
                                )
                            elif r < 3:
                                ya = yacc_tiles[(st, db)]
                                nc.vector.tensor_tensor(
                                    ya[:, :], py[:, :], ya[:, :], add,
                                )
                            else:
                                ya = yacc_tiles[(st, db)]
                                y_sb = ypool.tile([128, 512], F32, tag="y")
                                nc.vector.tensor_tensor(
                                    y_sb[:, :], py[:, :], ya[:, :], add,
                                )
                                nc.sync.dma_start(
                                    out=y[st * 128:(st + 1) * 128,
                                          db * 512:(db + 1) * 512],
                                    in_=y_sb[:, :],
                                )

                def wo_fetch(h4):
                    for hh in range(h4, h4 + 4):
                        for db in range(4):
                            wt = wopool.tile([128, 512], F16, tag="wo")
                            nc.sync.dma_start(
                                out=wt[:, :], in_=Wo[:, hh, db, :]
                            )
                            wo_tiles[(hh, db)] = wt

                wq_tiles = {}

                def wq_fetch(hh):
                    wq_t = wqpool.tile([128, ND, 128], F16, tag="wq")
                    nc.sync.dma_start(out=wq_t[:, :, :], in_=Wq[:, hh, :, :])
                    wq_tiles[hh] = wq_t

                for hh in range(3):
                    wq_fetch(hh)
                wo_fetch(0)

                for h in range(HQ):
                    pp, hi = divmod(h, 2)
                    if h + 3 < HQ:
                        wq_fetch(h + 3)
                    wq_t = wq_tiles.pop(h)

                    pq = pspool.tile([128, 2, SBLK], F32, tag="sc")
                    for d in range(ND):
                        nc.tensor.matmul(
                            pq[:, 0, :],
                            lhsT=wq_t[:, d, :],
                            rhs=xq[:, d, :],
                            start=(d == 0), stop=(d == ND - 1),
                        )
                    qT = qtpool.tile([128, SBLK], F16, tag="qt")
                    nc.vector.tensor_scalar(
                        qT[:, :], pq[:, 0, :], bq_cols[:, h:h + 1], None, add
                    )

                    if hi == 0:
                        outT = otpool.tile([128, 2, SBLK], F16, tag="ot")
                        outT_pairs.append(outT)
                    outT = outT_pairs[pp]

                    psum_o = popool.tile([128, SBLK], F32, tag="po")
                    dacc = dacpool.tile([128, 2, SBLK], F16, tag="da")
                    for tp in range(NT // 2):
                        psc = pspool.tile([128, 2, SBLK], F32, tag="sc")
                        for half in range(2):
                            t = tp * 2 + half
                            nc.tensor.matmul(
                                psc[:, half, :],
                                lhsT=kT[:, t * 128:(t + 1) * 128],
                                rhs=qT[:, :],
                                start=True, stop=True,
                            )
                        pT = ptpool.tile([128, 2, SBLK], F16, tag="pT")
                        nc.scalar.activation(
                            pT[:, :, :], psc[:, :, :], Exp,
                            bias=ebias_col[:, :], scale=SCALE,
                        )
                        for half in range(2):
                            t = tp * 2 + half
                            nc.tensor.matmul(
                                psum_o[:, :],
                                lhsT=v_nat[:, t, :],
                                rhs=pT[:, half, :],
                                start=(t == 0), stop=(t == NT - 1),
                            )
                        # denominator: elementwise-accumulate p on DVE
                        if tp == 0:
                            nc.vector.tensor_copy(dacc[:, :, :], pT[:, :, :])
                        else:
                            nc.vector.tensor_tensor(
                                dacc[:, :, :], dacc[:, :, :], pT[:, :, :], add
                            )

                    paux = auxpool.tile([128, SBLK], F32, tag="aux")
                    psum_den = paux
                    for half in range(2):
                        nc.tensor.matmul(
                            psum_den[0:1, :],
                            lhsT=ones_col,
                            rhs=dacc[:, half, :],
                            start=(half == 0), stop=(half == 1),
                        )
                    recip_f = rcpool.tile([1, SBLK], F32, tag="rcf")
                    nc.vector.reciprocal_approx_fast(
                        recip_f[0:1, :], psum_den[0:1, :]
                    )
                    recip = rcpool.tile([1, SBLK], F32R, tag="rc")
                    nc.vector.tensor_copy(recip[0:1, :], recip_f[0:1, :])
                    pb = paux   # reuse the bank: den row was consumed by recip
                    nc.tensor.matmul(
                        pb[:, :], lhsT=ones_row[0:1, :],
                        rhs=recip[0:1, :],
                        start=True, stop=True,
                    )
                    recip_b = rcpool.tile([128, SBLK], F32, tag="rb")
                    nc.scalar.copy(recip_b[:, :], pb[:, :])
                    nc.vector.tensor_tensor(
                        outT[:, hi, :], psum_o[:, :], recip_b[:, :], mult
                    )

                    # overlap Wo DMA + rounds with the attention loop
                    if h == 3:
                        wo_round(0)
                        wo_fetch(4)
                    elif h == 7:
                        wo_round(1)
                        wo_fetch(8)
                    elif h == 11:
                        wo_round(2)
                        wo_fetch(12)

                wo_round(3)

    nc.compile()
    return nc


def _get_nc():
    if "nc" not in _cache:
        _cache["nc"] = _build()
    return _cache["nc"]


def _prepare_in_maps(x, Wq, bq, Wk, bk, Wv, bv, Wo, bo):
    x = np.asarray(x, dtype=np.float32)
    bq = np.asarray(bq, dtype=np.float32)
    bk = np.asarray(bk, dtype=np.float32)
    bv = np.asarray(bv, dtype=np.float32)

    Wq = np.asarray(Wq, np.float32)
    Wk = np.asarray(Wk, np.float32)
    Wv = np.asarray(Wv, np.float32)
    Wo = np.asarray(Wo, np.float32)

    # [p, h, n, m] <- Wq[n*128+p, h*128+m]  (contiguous 2KB DMA lines)
    Wq_pre = _to_f16(Wq.reshape(ND, 128, HQ, 128).transpose(1, 2, 0, 3))
    Wk_pre = _to_f16(Wk.reshape(ND, 128, DH).transpose(1, 0, 2))
    Wv_pre = _to_f16(Wv.reshape(ND, 128, DH).transpose(1, 0, 2))
    # [p, h, db, m] <- Wo[h*128+p, db*512+m]
    Wo_pre = _to_f16(Wo.reshape(HQ, 128, 4, 512).transpose(1, 0, 2, 3))
    bo_pre = _round_fp32r(np.asarray(bo, np.float32)).reshape(1, D)

    ones16 = np.ones((128, 16), np.float16)
    onesr = np.ones((1, 128), np.float32)

    # xT_pre[g]: [p, n, s] <- x[g].T[n*128+p, s]
    xT_pre = [
        _to_f16(x[g].T.reshape(ND, 128, S).transpose(1, 0, 2))
        for g in range(B)
    ]
    in_maps = []
    for c in range(N_CORES):
        g, blk = divmod(c, 4)
        s0 = blk * SBLK
        in_maps.append({
            "xall": xT_pre[g],
            "xq": np.ascontiguousarray(xT_pre[g][:, :, s0:s0 + SBLK]),
            "Wq": Wq_pre, "bq": bq, "Wk": Wk_pre, "bk": bk,
            "Wv": Wv_pre, "bv": bv, "Wo": Wo_pre, "bo": bo_pre,
            "ones16": ones16, "onesr": onesr,
        })
    return in_maps


def _assemble(results):
    out = np.empty((B, S, D), dtype=np.float32)
    for c in range(N_CORES):
        g, blk = divmod(c, 4)
        out[g, blk * SBLK:(blk + 1) * SBLK, :] = results[c]["y"]
    return out


def kernel(x, Wq, bq, Wk, bk, Wv, bv, Wo, bo):
    from concourse.bass_utils import run_bass_kernel_spmd

    in_maps = _prepare_in_maps(x, Wq, bq, Wk, bk, Wv, bv, Wo, bo)
    nc = _get_nc()
    res = run_bass_kernel_spmd(nc, in_maps, core_ids=list(range(N_CORES)))
    return _assemble(res.results)


# revision 21
# speedup vs baseline: 1.3742x; 1.0388x over previous
"""Multi-head attention block (16 query heads, shared single K/V head) on
8 Trainium2 NeuronCores — fp16 pipeline.

Reference computation (B=2, S=2048, D=2048, HQ=16, DH=128, fp32):
    q = (x @ Wq + bq)  -> [B, S, 16, 128]
    k = x @ Wk + bk    -> [B, S, 128]   (single shared K/V head)
    v = x @ Wv + bv    -> [B, S, 128]
    attn = softmax(q k^T / sqrt(128))
    out = (attn @ v) reshaped -> [B, S, D];  y = out @ Wo + bo
    (dropout is identity in eval)

Sharding: batch x sequence-block data parallel (no collectives). Core c
handles batch c//4, query rows (c%4)*512 .. +512, for ALL 16 heads; K/V
over the full sequence are recomputed per core (cheap).

Precision: the attention output is a softmax-weighted mean over ~750
effective keys, so its magnitude is ~27x smaller than v's; quantization
noise on any matmul operand passes through to the output at roughly its
per-element RMS. fp8's ~4%/element is far too coarse, so every operand
runs fp16 (~0.1%/element, full PE rate, fast-weight-load eligible) with
fp32 PSUM accumulation. Measured end-to-end error vs the fp32 reference
is ~2e-3.

Structure: scores stay in the transposed [key, query] layout end-to-end
(softmax skips max-subtraction; scores ~N(0,1) by construction, and exp
applies a constant -3 offset that cancels in the normalization). exp runs
on ScalarE straight PSUM->SBUF; p@v contracts the key axis on the PE with
no transposes. Softmax denominators come from DVE tile-accumulation of p
plus one tiny ones-matmul per head (saving ~120k PE cycles vs per-tile
ones-matmuls); the per-head normalization uses reciprocal_approx_fast and
a PE row-broadcast. The Wo projection runs in 4 rounds of 4 heads so 3/4
of it overlaps the attention loop.
"""

import numpy as np

B, S, D = 2, 2048, 2048
HQ, DH = 16, 128
SBLK = S // 4          # 512 query rows per core
N_CORES = 8
SCALE = 1.0 / float(np.sqrt(DH))
EXP_BIAS = -3.0        # exp(s/sqrt(DH) - 3): cancels in softmax

ND = D // 128          # 16 contraction chunks
NT = S // 128          # 16 key tiles
NQ = SBLK // 128       # 4 query row-tiles per core

_cache = {}


def _round_fp32r(a):
    """Round fp32 to fp32r (1s+8e+11m) with round-to-nearest-even-ish."""
    b = np.ascontiguousarray(a, dtype=np.float32).view(np.uint32)
    bias = np.uint32(0x7FF) + ((b >> np.uint32(12)) & np.uint32(1))
    return ((b + bias) & np.uint32(0xFFFFF000)).view(np.float32)


def _to_f16(a):
    return np.ascontiguousarray(np.asarray(a, np.float32)).astype(np.float16)


def _build():
    from concourse import bacc, mybir, tile
    from concourse.masks import make_identity

    F32 = mybir.dt.float32
    F32R = mybir.dt.float32r
    F16 = mybir.dt.float16
    Exp = mybir.ActivationFunctionType.Exp
    mult = mybir.AluOpType.mult
    add = mybir.AluOpType.add

    nc = bacc.Bacc("TRN2", target_bir_lowering=False, debug=False,
                   num_devices=N_CORES)

    # pre-rearranged on host; see _prepare_in_maps
    xall_d = nc.dram_tensor("xall", [128, ND, S], F16, kind="ExternalInput").ap()
    xq_d = nc.dram_tensor("xq", [128, ND, SBLK], F16, kind="ExternalInput").ap()
    Wq = nc.dram_tensor("Wq", [128, HQ, ND, 128], F16, kind="ExternalInput").ap()
    bq = nc.dram_tensor("bq", [D], F32, kind="ExternalInput").ap()
    Wk = nc.dram_tensor("Wk", [128, ND, DH], F16, kind="ExternalInput").ap()
    bk = nc.dram_tensor("bk", [DH], F32, kind="ExternalInput").ap()
    Wv = nc.dram_tensor("Wv", [128, ND, DH], F16, kind="ExternalInput").ap()
    bv = nc.dram_tensor("bv", [DH], F32, kind="ExternalInput").ap()
    Wo = nc.dram_tensor("Wo", [128, HQ, 4, 512], F16, kind="ExternalInput").ap()
    bo = nc.dram_tensor("bo", [1, D], F32R, kind="ExternalInput").ap()
    ones16_d = nc.dram_tensor("ones16", [128, 16], F16, kind="ExternalInput").ap()
    onesr_d = nc.dram_tensor("onesr", [1, 128], F32R, kind="ExternalInput").ap()
    y = nc.dram_tensor("y", [SBLK, D], F32, kind="ExternalOutput").ap()

    with tile.TileContext(nc) as tc, nc.allow_low_precision(
        reason="fp16 matmul pipeline; verified against fp32 reference"
    ):
        with (
            tc.tile_pool(name="const", bufs=1) as cpool,
            tc.tile_pool(name="live", bufs=1) as lpool,
            tc.tile_pool(name="ot", bufs=HQ // 2) as otpool,  # 8 head-pair outs
            tc.tile_pool(name="ya", bufs=16) as yapool,       # y accumulators
            tc.tile_pool(name="wo", bufs=20) as wopool,
            tc.tile_pool(name="yw", bufs=3) as ypool,
            tc.tile_pool(name="rc", bufs=2) as rcpool,
        ):
            # ---- constants -------------------------------------------------
            ones16 = cpool.tile([128, 16], F16)
            nc.sync.dma_start(out=ones16[:, :], in_=ones16_d[:, :])
            ones_col = ones16[:, 0:1]
            ones_row = cpool.tile([1, 128], F32R)
            nc.sync.dma_start(out=ones_row[:, :], in_=onesr_d[:, :])
            ident = cpool.tile([128, 128], F32)
            make_identity(nc, ident[:, :])
            ebias_col = cpool.tile([128, 1], F32)
            nc.gpsimd.memset(ebias_col[:, :], EXP_BIAS)

            bk_col = cpool.tile([128, 1], F32)
            nc.sync.dma_start(out=bk_col[:, :], in_=bk[:].unsqueeze(1))
            bv_col = cpool.tile([128, 1], F32)
            nc.sync.dma_start(out=bv_col[:, :], in_=bv[:].unsqueeze(1))
            bq_cols = cpool.tile([128, HQ], F32)
            nc.sync.dma_start(
                out=bq_cols[:, :], in_=bq[:].rearrange("(h p) -> p h", p=128)
            )
            bo_row = cpool.tile([1, D], F32R)
            nc.sync.dma_start(out=bo_row[:, :], in_=bo[:, :])

            xq = lpool.tile([128, ND, SBLK], F16)
            kT = lpool.tile([128, S], F16)
            v_nat = lpool.tile([128, NT, DH], F16)

            # ---- phase A: k/v projections over the full sequence -----------
            # xall is scoped here so its 64KB/partition frees before phase B.
            with (
                tc.tile_pool(name="pha", bufs=1) as apool,
                tc.tile_pool(name="pacc", bufs=1, space="PSUM") as pacc,
                tc.tile_pool(name="ptr", bufs=2, space="PSUM") as ptrp,
            ):
                wk_all = apool.tile([128, ND, DH], F16)
                nc.sync.dma_start(out=wk_all[:, :, :], in_=Wk[:, :, :])
                wv_all = apool.tile([128, ND, DH], F16)
                nc.sync.dma_start(out=wv_all[:, :, :], in_=Wv[:, :, :])
                xall = apool.tile([128, ND, S], F16)
                for c8 in range(8):
                    nc.sync.dma_start(
                        out=xall[:, c8 * 2:(c8 + 1) * 2, :],
                        in_=xall_d[:, c8 * 2:(c8 + 1) * 2, :],
                    )
                    if c8 == 0:
                        nc.sync.dma_start(out=xq[:, :, :], in_=xq_d[:, :, :])
                vT = apool.tile([128, S], F32)

                HS = S // 2
                for th in range(2):
                    tsl = slice(th * HS, (th + 1) * HS)
                    psum_k = pacc.tile([128, HS], F32, tag="pk")
                    psum_v = pacc.tile([128, HS], F32, tag="pv")
                    for d in range(ND):
                        for nb in range(HS // 512):
                            sl = slice(nb * 512, (nb + 1) * 512)
                            xsl = slice(th * HS + nb * 512,
                                        th * HS + (nb + 1) * 512)
                            nc.tensor.matmul(
                                psum_k[:, sl],
                                lhsT=wk_all[:, d, :],
                                rhs=xall[:, d, xsl],
                                start=(d == 0), stop=(d == ND - 1),
                            )
                            nc.tensor.matmul(
                                psum_v[:, sl],
                                lhsT=wv_all[:, d, :],
                                rhs=xall[:, d, xsl],
                                start=(d == 0), stop=(d == ND - 1),
                            )

                    nc.vector.tensor_scalar(
                        kT[:, tsl], psum_k[:, :], bk_col[:, :], None, add
                    )
                    nc.vector.tensor_scalar(
                        vT[:, tsl], psum_v[:, :], bv_col[:, :], None, add
                    )
                    # v into natural [key, DH] layout for p@v; th0's half is
                    # emitted after th1's matmuls so the in-order PE queue
                    # isn't head-of-line blocked waiting on vT's bias-add.
                    if th == 1:
                        for t in range(NT // 2):
                            ptr = ptrp.tile([128, 128], F32, tag="tr")
                            nc.tensor.transpose(
                                ptr[:, :], vT[:, t * 128:(t + 1) * 128],
                                ident[:, :],
                            )
                            nc.vector.tensor_copy(v_nat[:, t, :], ptr[:, :])

                for t in range(NT // 2, NT):
                    ptr = ptrp.tile([128, 128], F32, tag="tr")
                    nc.tensor.transpose(
                        ptr[:, :], vT[:, t * 128:(t + 1) * 128], ident[:, :]
                    )
                    nc.vector.tensor_copy(v_nat[:, t, :], ptr[:, :])

            # ---- phase B (attention) + phase C (Wo) interleaved ------------
            outT_pairs = []
            yacc_tiles = {}
            wo_tiles = {}

            with (
                tc.tile_pool(name="wq", bufs=3) as wqpool,
                tc.tile_pool(name="qt", bufs=2) as qtpool,
                tc.tile_pool(name="pt", bufs=3) as ptpool,
                tc.tile_pool(name="dac", bufs=2) as dacpool,
                tc.tile_pool(name="ps", bufs=2, space="PSUM") as pspool,
                tc.tile_pool(name="po", bufs=2, space="PSUM") as popool,
                tc.tile_pool(name="aux", bufs=1, space="PSUM") as auxpool,
                tc.tile_pool(name="py", bufs=1, space="PSUM") as pypool,
            ):
                # bo broadcast [1,D] -> [128,D] via PE
                bo_b = cpool.tile([128, D], F32)
                for nb in range(D // 512):
                    pbo = pypool.tile([128, 512], F32, tag="py")
                    nc.tensor.matmul(
                        pbo[:, :], lhsT=ones_row[0:1, :],
                        rhs=bo_row[:, nb * 512:(nb + 1) * 512],
                        start=True, stop=True,
                    )
                    nc.scalar.copy(bo_b[:, nb * 512:(nb + 1) * 512], pbo[:, :])

                def wo_round(heads, first=False, final=False):
                    """Accumulate the given heads into the y accumulators."""
                    for st in range(NQ):
                        for db in range(4):
                            py = pypool.tile([128, 512], F32, tag="py")
                            for j, h in enumerate(heads):
                                pp, hi = divmod(h, 2)
                                nc.tensor.matmul(
                                    py[:, :],
                                    lhsT=outT_pairs[pp][
                                        :, hi, st * 128:(st + 1) * 128],
                                    rhs=wo_tiles[(h, db)][:, :],
                                    start=(j == 0), stop=(j == len(heads) - 1),
                                )
                            if first:
                                ya = yapool.tile([128, 512], F32, tag="ya")
                                yacc_tiles[(st, db)] = ya
                                nc.vector.tensor_tensor(
                                    ya[:, :], py[:, :],
                                    bo_b[:, db * 512:(db + 1) * 512], add,
                                )
                            elif not final:
                                ya = yacc_tiles[(st, db)]
                                nc.vector.tensor_tensor(
                                    ya[:, :], py[:, :], ya[:, :], add,
                                )
                            else:
                                ya = yacc_tiles[(st, db)]
                                y_sb = ypool.tile([128, 512], F32, tag="y")
                                nc.vector.tensor_tensor(
                                    y_sb[:, :], py[:, :], ya[:, :], add,
                                )
                                nc.sync.dma_start(
                                    out=y[st * 128:(st + 1) * 128,
                                          db * 512:(db + 1) * 512],
                                    in_=y_sb[:, :],
                                )

                def wo_fetch(h4):
                    for hh in range(h4, h4 + 4):
                        for db in range(4):
                            wt = wopool.tile([128, 512], F16, tag="wo")
                            nc.sync.dma_start(
                                out=wt[:, :], in_=Wo[:, hh, db, :]
                            )
                            wo_tiles[(hh, db)] = wt

                wq_tiles = {}

                def wq_fetch(hh):
                    wq_t = wqpool.tile([128, ND, 128], F16, tag="wq")
                    nc.sync.dma_start(out=wq_t[:, :, :], in_=Wq[:, hh, :, :])
                    wq_tiles[hh] = wq_t

                for hh in range(3):
                    wq_fetch(hh)
                wo_fetch(0)

                for h in range(HQ):
                    pp, hi = divmod(h, 2)
                    if h + 3 < HQ:
                        wq_fetch(h + 3)
                    wq_t = wq_tiles.pop(h)

                    pq = pspool.tile([128, 2, SBLK], F32, tag="sc")
                    for d in range(ND):
                        nc.tensor.matmul(
                            pq[:, 0, :],
                            lhsT=wq_t[:, d, :],
                            rhs=xq[:, d, :],
                            start=(d == 0), stop=(d == ND - 1),
                        )
                    qT = qtpool.tile([128, SBLK], F16, tag="qt")
                    nc.vector.tensor_scalar(
                        qT[:, :], pq[:, 0, :], bq_cols[:, h:h + 1], None, add
                    )

                    if hi == 0:
                        outT = otpool.tile([128, 2, SBLK], F16, tag="ot")
                        outT_pairs.append(outT)
                    outT = outT_pairs[pp]

                    psum_o = popool.tile([128, SBLK], F32, tag="po")
                    dacc = dacpool.tile([128, 2, SBLK], F16, tag="da")
                    for tp in range(NT // 2):
                        psc = pspool.tile([128, 2, SBLK], F32, tag="sc")
                        for half in range(2):
                            t = tp * 2 + half
                            nc.tensor.matmul(
                                psc[:, half, :],
                                lhsT=kT[:, t * 128:(t + 1) * 128],
                                rhs=qT[:, :],
                                start=True, stop=True,
                            )
                        pT = ptpool.tile([128, 2, SBLK], F16, tag="pT")
                        nc.scalar.activation(
                            pT[:, :, :], psc[:, :, :], Exp,
                            bias=ebias_col[:, :], scale=SCALE,
                        )
                        for half in range(2):
                            t = tp * 2 + half
                            nc.tensor.matmul(
                                psum_o[:, :],
                                lhsT=v_nat[:, t, :],
                                rhs=pT[:, half, :],
                                start=(t == 0), stop=(t == NT - 1),
                            )
                        # denominator: elementwise-accumulate p on DVE
                        if tp == 0:
                            nc.vector.tensor_copy(dacc[:, :, :], pT[:, :, :])
                        else:
                            nc.vector.tensor_tensor(
                                dacc[:, :, :], dacc[:, :, :], pT[:, :, :], add
                            )

                    paux = auxpool.tile([128, SBLK], F32, tag="aux")
                    psum_den = paux
                    for half in range(2):
                        nc.tensor.matmul(
                            psum_den[0:1, :],
                            lhsT=ones_col,
                            rhs=dacc[:, half, :],
                            start=(half == 0), stop=(half == 1),
                        )
                    recip_f = rcpool.tile([1, SBLK], F32, tag="rcf")
                    nc.vector.reciprocal_approx_fast(
                        recip_f[0:1, :], psum_den[0:1, :]
                    )
                    recip = rcpool.tile([1, SBLK], F32R, tag="rc")
                    nc.vector.tensor_copy(recip[0:1, :], recip_f[0:1, :])
                    pb = paux   # reuse the bank: den row was consumed by recip
                    nc.tensor.matmul(
                        pb[:, :], lhsT=ones_row[0:1, :],
                        rhs=recip[0:1, :],
                        start=True, stop=True,
                    )
                    recip_b = rcpool.tile([128, SBLK], F32, tag="rb")
                    nc.scalar.copy(recip_b[:, :], pb[:, :])
                    nc.vector.tensor_tensor(
                        outT[:, hi, :], psum_o[:, :], recip_b[:, :], mult
                    )

                    # overlap Wo DMA + rounds with the attention loop
                    if h == 3:
                        wo_round([0, 1, 2, 3], first=True)
                        wo_fetch(4)
                    elif h == 7:
                        wo_round([4, 5, 6, 7])
                        wo_fetch(8)
                    elif h == 11:
                        wo_round([8, 9, 10, 11])
                        wo_fetch(12)
                    elif h == 13:
                        wo_round([12, 13])

                wo_round([14, 15], final=True)

    nc.compile()
    return nc


def _get_nc():
    if "nc" not in _cache:
        _cache["nc"] = _build()
    return _cache["nc"]


def _prepare_in_maps(x, Wq, bq, Wk, bk, Wv, bv, Wo, bo):
    x = np.asarray(x, dtype=np.float32)
    bq = np.asarray(bq, dtype=np.float32)
    bk = np.asarray(bk, dtype=np.float32)
    bv = np.asarray(bv, dtype=np.float32)

    Wq = np.asarray(Wq, np.float32)
    Wk = np.asarray(Wk, np.float32)
    Wv = np.asarray(Wv, np.float32)
    Wo = np.asarray(Wo, np.float32)

    # [p, h, n, m] <- Wq[n*128+p, h*128+m]  (contiguous 2KB DMA lines)
    Wq_pre = _to_f16(Wq.reshape(ND, 128, HQ, 128).transpose(1, 2, 0, 3))
    Wk_pre = _to_f16(Wk.reshape(ND, 128, DH).transpose(1, 0, 2))
    Wv_pre = _to_f16(Wv.reshape(ND, 128, DH).transpose(1, 0, 2))
    # [p, h, db, m] <- Wo[h*128+p, db*512+m]
    Wo_pre = _to_f16(Wo.reshape(HQ, 128, 4, 512).transpose(1, 0, 2, 3))
    bo_pre = _round_fp32r(np.asarray(bo, np.float32)).reshape(1, D)

    ones16 = np.ones((128, 16), np.float16)
    onesr = np.ones((1, 128), np.float32)

    # xT_pre[g]: [p, n, s] <- x[g].T[n*128+p, s]
    xT_pre = [
        _to_f16(x[g].T.reshape(ND, 128, S).transpose(1, 0, 2))
        for g in range(B)
    ]
    in_maps = []
    for c in range(N_CORES):
        g, blk = divmod(c, 4)
        s0 = blk * SBLK
        in_maps.append({
            "xall": xT_pre[g],
            "xq": np.ascontiguousarray(xT_pre[g][:, :, s0:s0 + SBLK]),
            "Wq": Wq_pre, "bq": bq, "Wk": Wk_pre, "bk": bk,
            "Wv": Wv_pre, "bv": bv, "Wo": Wo_pre, "bo": bo_pre,
            "ones16": ones16, "onesr": onesr,
        })
    return in_maps


def _assemble(results):
    out = np.empty((B, S, D), dtype=np.float32)
    for c in range(N_CORES):
        g, blk = divmod(c, 4)
        out[g, blk * SBLK:(blk + 1) * SBLK, :] = results[c]["y"]
    return out


def kernel(x, Wq, bq, Wk, bk, Wv, bv, Wo, bo):
    from concourse.bass_utils import run_bass_kernel_spmd

    in_maps = _prepare_in_maps(x, Wq, bq, Wk, bk, Wv, bv, Wo, bo)
    nc = _get_nc()
    res = run_bass_kernel_spmd(nc, in_maps, core_ids=list(range(N_CORES)))
    return _assemble(res.results)


# revision 22
# speedup vs baseline: 1.4143x; 1.0292x over previous
"""Multi-head attention block (16 query heads, shared single K/V head) on
8 Trainium2 NeuronCores — fp16 pipeline.

Reference computation (B=2, S=2048, D=2048, HQ=16, DH=128, fp32):
    q = (x @ Wq + bq)  -> [B, S, 16, 128]
    k = x @ Wk + bk    -> [B, S, 128]   (single shared K/V head)
    v = x @ Wv + bv    -> [B, S, 128]
    attn = softmax(q k^T / sqrt(128))
    out = (attn @ v) reshaped -> [B, S, D];  y = out @ Wo + bo
    (dropout is identity in eval)

Sharding: batch x sequence-block data parallel (no collectives). Core c
handles batch c//4, query rows (c%4)*512 .. +512, for ALL 16 heads; K/V
over the full sequence are recomputed per core (cheap).

Precision: the attention output is a softmax-weighted mean over ~750
effective keys, so its magnitude is ~27x smaller than v's; quantization
noise on any matmul operand passes through to the output at roughly its
per-element RMS. fp8's ~4%/element is far too coarse, so every operand
runs fp16 (~0.1%/element, full PE rate, fast-weight-load eligible) with
fp32 PSUM accumulation. Measured end-to-end error vs the fp32 reference
is ~2e-3.

Structure: scores stay in the transposed [key, query] layout end-to-end
(softmax skips max-subtraction; scores ~N(0,1) by construction, and exp
applies a constant -3 offset that cancels in the normalization). exp runs
on ScalarE straight PSUM->SBUF; p@v contracts the key axis on the PE with
no transposes. Softmax denominators come from DVE tile-accumulation of p
plus one tiny ones-matmul per head (saving ~120k PE cycles vs per-tile
ones-matmuls); the per-head normalization uses reciprocal_approx_fast and
a PE row-broadcast. The Wo projection runs in 4 rounds of 4 heads so 3/4
of it overlaps the attention loop.
"""

import numpy as np

B, S, D = 2, 2048, 2048
HQ, DH = 16, 128
SBLK = S // 4          # 512 query rows per core
N_CORES = 8
SCALE = 1.0 / float(np.sqrt(DH))
EXP_BIAS = -3.0        # exp(s/sqrt(DH) - 3): cancels in softmax

ND = D // 128          # 16 contraction chunks
NT = S // 128          # 16 key tiles
NQ = SBLK // 128       # 4 query row-tiles per core

_cache = {}


def _round_fp32r(a):
    """Round fp32 to fp32r (1s+8e+11m) with round-to-nearest-even-ish."""
    b = np.ascontiguousarray(a, dtype=np.float32).view(np.uint32)
    bias = np.uint32(0x7FF) + ((b >> np.uint32(12)) & np.uint32(1))
    return ((b + bias) & np.uint32(0xFFFFF000)).view(np.float32)


def _to_f16(a):
    return np.ascontiguousarray(np.asarray(a, np.float32)).astype(np.float16)


def _build():
    from concourse import bacc, mybir, tile
    from concourse.masks import make_identity

    F32 = mybir.dt.float32
    F32R = mybir.dt.float32r
    F16 = mybir.dt.float16
    Exp = mybir.ActivationFunctionType.Exp
    mult = mybir.AluOpType.mult
    add = mybir.AluOpType.add

    nc = bacc.Bacc("TRN2", target_bir_lowering=False, debug=False,
                   num_devices=N_CORES)

    # pre-rearranged on host; see _prepare_in_maps
    xall_d = nc.dram_tensor("xall", [128, ND, S], F16, kind="ExternalInput").ap()
    xq_d = nc.dram_tensor("xq", [128, ND, SBLK], F16, kind="ExternalInput").ap()
    Wq = nc.dram_tensor("Wq", [128, HQ, ND, 128], F16, kind="ExternalInput").ap()
    bq = nc.dram_tensor("bq", [D], F32, kind="ExternalInput").ap()
    Wk = nc.dram_tensor("Wk", [128, ND, DH], F16, kind="ExternalInput").ap()
    bk = nc.dram_tensor("bk", [DH], F32, kind="ExternalInput").ap()
    Wv = nc.dram_tensor("Wv", [128, ND, DH], F16, kind="ExternalInput").ap()
    bv = nc.dram_tensor("bv", [DH], F32, kind="ExternalInput").ap()
    Wo = nc.dram_tensor("Wo", [128, HQ, 4, 512], F16, kind="ExternalInput").ap()
    bo = nc.dram_tensor("bo", [1, D], F32R, kind="ExternalInput").ap()
    ones16_d = nc.dram_tensor("ones16", [128, 16], F16, kind="ExternalInput").ap()
    onesr_d = nc.dram_tensor("onesr", [1, 128], F32R, kind="ExternalInput").ap()
    y = nc.dram_tensor("y", [SBLK, D], F32, kind="ExternalOutput").ap()

    with tile.TileContext(nc) as tc, nc.allow_low_precision(
        reason="fp16 matmul pipeline; verified against fp32 reference"
    ):
        with (
            tc.tile_pool(name="const", bufs=1) as cpool,
            tc.tile_pool(name="live", bufs=1) as lpool,
            tc.tile_pool(name="ot", bufs=HQ // 2) as otpool,  # 8 head-pair outs
            tc.tile_pool(name="ya", bufs=16) as yapool,       # y accumulators
            tc.tile_pool(name="wo", bufs=20) as wopool,
            tc.tile_pool(name="yw", bufs=3) as ypool,
            tc.tile_pool(name="rc", bufs=2) as rcpool,
        ):
            # ---- constants -------------------------------------------------
            ones16 = cpool.tile([128, 16], F16)
            nc.sync.dma_start(out=ones16[:, :], in_=ones16_d[:, :])
            ones_col = ones16[:, 0:1]
            ones_row = cpool.tile([1, 128], F32R)
            nc.sync.dma_start(out=ones_row[:, :], in_=onesr_d[:, :])
            ident = cpool.tile([128, 128], F32)
            make_identity(nc, ident[:, :])
            ebias_col = cpool.tile([128, 1], F32)
            nc.gpsimd.memset(ebias_col[:, :], EXP_BIAS)

            bk_col = cpool.tile([128, 1], F32)
            nc.sync.dma_start(out=bk_col[:, :], in_=bk[:].unsqueeze(1))
            bv_col = cpool.tile([128, 1], F32)
            nc.sync.dma_start(out=bv_col[:, :], in_=bv[:].unsqueeze(1))
            bq_cols = cpool.tile([128, HQ], F32)
            nc.sync.dma_start(
                out=bq_cols[:, :], in_=bq[:].rearrange("(h p) -> p h", p=128)
            )
            bo_row = cpool.tile([1, D], F32R)
            nc.sync.dma_start(out=bo_row[:, :], in_=bo[:, :])

            xq = lpool.tile([128, ND, SBLK], F16)
            kT = lpool.tile([128, S], F16)
            v_nat = lpool.tile([128, NT, DH], F16)

            # ---- phase A: k/v projections over the full sequence -----------
            # xall is scoped here so its 64KB/partition frees before phase B.
            with (
                tc.tile_pool(name="pha", bufs=1) as apool,
                tc.tile_pool(name="pacc", bufs=1, space="PSUM") as pacc,
                tc.tile_pool(name="ptr", bufs=2, space="PSUM") as ptrp,
            ):
                wk_all = apool.tile([128, ND, DH], F16)
                nc.sync.dma_start(out=wk_all[:, :, :], in_=Wk[:, :, :])
                wv_all = apool.tile([128, ND, DH], F16)
                nc.sync.dma_start(out=wv_all[:, :, :], in_=Wv[:, :, :])
                xall = apool.tile([128, ND, S], F16)
                for c8 in range(8):
                    nc.sync.dma_start(
                        out=xall[:, c8 * 2:(c8 + 1) * 2, :],
                        in_=xall_d[:, c8 * 2:(c8 + 1) * 2, :],
                    )
                    if c8 == 0:
                        nc.sync.dma_start(out=xq[:, :, :], in_=xq_d[:, :, :])
                vT = apool.tile([128, S], F32)

                HS = S // 2
                for th in range(2):
                    tsl = slice(th * HS, (th + 1) * HS)
                    psum_k = pacc.tile([128, HS], F32, tag="pk")
                    psum_v = pacc.tile([128, HS], F32, tag="pv")
                    for d in range(ND):
                        for nb in range(HS // 512):
                            sl = slice(nb * 512, (nb + 1) * 512)
                            xsl = slice(th * HS + nb * 512,
                                        th * HS + (nb + 1) * 512)
                            nc.tensor.matmul(
                                psum_k[:, sl],
                                lhsT=wk_all[:, d, :],
                                rhs=xall[:, d, xsl],
                                start=(d == 0), stop=(d == ND - 1),
                            )
                            nc.tensor.matmul(
                                psum_v[:, sl],
                                lhsT=wv_all[:, d, :],
                                rhs=xall[:, d, xsl],
                                start=(d == 0), stop=(d == ND - 1),
                            )

                    nc.vector.tensor_scalar(
                        kT[:, tsl], psum_k[:, :], bk_col[:, :], None, add
                    )
                    nc.vector.tensor_scalar(
                        vT[:, tsl], psum_v[:, :], bv_col[:, :], None, add
                    )
                    # v into natural [key, DH] layout for p@v; th0's half is
                    # emitted after th1's matmuls so the in-order PE queue
                    # isn't head-of-line blocked waiting on vT's bias-add.
                    if th == 1:
                        for t in range(NT // 2):
                            ptr = ptrp.tile([128, 128], F32, tag="tr")
                            nc.tensor.transpose(
                                ptr[:, :], vT[:, t * 128:(t + 1) * 128],
                                ident[:, :],
                            )
                            nc.vector.tensor_copy(v_nat[:, t, :], ptr[:, :])

                for t in range(NT // 2, NT):
                    ptr = ptrp.tile([128, 128], F32, tag="tr")
                    nc.tensor.transpose(
                        ptr[:, :], vT[:, t * 128:(t + 1) * 128], ident[:, :]
                    )
                    nc.vector.tensor_copy(v_nat[:, t, :], ptr[:, :])

            # ---- phase B (attention) + phase C (Wo) interleaved ------------
            outT_pairs = []
            yacc_tiles = {}
            wo_tiles = {}

            with (
                tc.tile_pool(name="wq", bufs=3) as wqpool,
                tc.tile_pool(name="qt", bufs=2) as qtpool,
                tc.tile_pool(name="pt", bufs=3) as ptpool,
                tc.tile_pool(name="dac", bufs=2) as dacpool,
                tc.tile_pool(name="ps", bufs=2, space="PSUM") as pspool,
                tc.tile_pool(name="po", bufs=2, space="PSUM") as popool,
                tc.tile_pool(name="aux", bufs=1, space="PSUM") as auxpool,
                tc.tile_pool(name="py", bufs=1, space="PSUM") as pypool,
            ):
                # bo broadcast [1,D] -> [128,D] via PE
                bo_b = cpool.tile([128, D], F32)
                for nb in range(D // 512):
                    pbo = pypool.tile([128, 512], F32, tag="py")
                    nc.tensor.matmul(
                        pbo[:, :], lhsT=ones_row[0:1, :],
                        rhs=bo_row[:, nb * 512:(nb + 1) * 512],
                        start=True, stop=True,
                    )
                    nc.scalar.copy(bo_b[:, nb * 512:(nb + 1) * 512], pbo[:, :])

                def wo_round(heads, first=False, final=False):
                    """Accumulate the given heads into the y accumulators."""
                    for st in range(NQ):
                        for db in range(4):
                            py = pypool.tile([128, 512], F32, tag="py")
                            for j, h in enumerate(heads):
                                pp, hi = divmod(h, 2)
                                nc.tensor.matmul(
                                    py[:, :],
                                    lhsT=outT_pairs[pp][
                                        :, hi, st * 128:(st + 1) * 128],
                                    rhs=wo_tiles[(h, db)][:, :],
                                    start=(j == 0), stop=(j == len(heads) - 1),
                                )
                            if first:
                                ya = yapool.tile([128, 512], F32, tag="ya")
                                yacc_tiles[(st, db)] = ya
                                nc.vector.tensor_tensor(
                                    ya[:, :], py[:, :],
                                    bo_b[:, db * 512:(db + 1) * 512], add,
                                )
                            elif not final:
                                ya = yacc_tiles[(st, db)]
                                nc.vector.tensor_tensor(
                                    ya[:, :], py[:, :], ya[:, :], add,
                                )
                            else:
                                ya = yacc_tiles[(st, db)]
                                y_sb = ypool.tile([128, 512], F32, tag="y")
                                nc.vector.tensor_tensor(
                                    y_sb[:, :], py[:, :], ya[:, :], add,
                                )
                                nc.sync.dma_start(
                                    out=y[st * 128:(st + 1) * 128,
                                          db * 512:(db + 1) * 512],
                                    in_=y_sb[:, :],
                                )

                def wo_fetch(h4):
                    for hh in range(h4, h4 + 4):
                        for db in range(4):
                            wt = wopool.tile([128, 512], F16, tag="wo")
                            nc.sync.dma_start(
                                out=wt[:, :], in_=Wo[:, hh, db, :]
                            )
                            wo_tiles[(hh, db)] = wt

                wq_tiles = {}

                def wq_fetch(hh):
                    wq_t = wqpool.tile([128, ND, 128], F16, tag="wq")
                    nc.sync.dma_start(out=wq_t[:, :, :], in_=Wq[:, hh, :, :])
                    wq_tiles[hh] = wq_t

                for hh in range(3):
                    wq_fetch(hh)
                wo_fetch(0)

                def qproj(hh):
                    """Q projection for head hh; emitted one head ahead so
                    these matmuls fill the PE while the previous head's qT
                    bias-add drains on DVE."""
                    wq_t = wq_tiles.pop(hh)
                    pq = pspool.tile([128, 2, SBLK], F32, tag="sc")
                    for d in range(ND):
                        nc.tensor.matmul(
                            pq[:, 0, :],
                            lhsT=wq_t[:, d, :],
                            rhs=xq[:, d, :],
                            start=(d == 0), stop=(d == ND - 1),
                        )
                    qT = qtpool.tile([128, SBLK], F16, tag="qt")
                    nc.vector.tensor_scalar(
                        qT[:, :], pq[:, 0, :], bq_cols[:, hh:hh + 1], None, add
                    )
                    return qT

                qT_next = qproj(0)

                for h in range(HQ):
                    pp, hi = divmod(h, 2)
                    if h + 3 < HQ:
                        wq_fetch(h + 3)
                    qT = qT_next
                    if h + 1 < HQ:
                        qT_next = qproj(h + 1)

                    if hi == 0:
                        outT = otpool.tile([128, 2, SBLK], F16, tag="ot")
                        outT_pairs.append(outT)
                    outT = outT_pairs[pp]

                    psum_o = popool.tile([128, SBLK], F32, tag="po")
                    dacc = dacpool.tile([128, 2, SBLK], F16, tag="da")
                    for tp in range(NT // 2):
                        psc = pspool.tile([128, 2, SBLK], F32, tag="sc")
                        for half in range(2):
                            t = tp * 2 + half
                            nc.tensor.matmul(
                                psc[:, half, :],
                                lhsT=kT[:, t * 128:(t + 1) * 128],
                                rhs=qT[:, :],
                                start=True, stop=True,
                            )
                        pT = ptpool.tile([128, 2, SBLK], F16, tag="pT")
                        nc.scalar.activation(
                            pT[:, :, :], psc[:, :, :], Exp,
                            bias=ebias_col[:, :], scale=SCALE,
                        )
                        for half in range(2):
                            t = tp * 2 + half
                            nc.tensor.matmul(
                                psum_o[:, :],
                                lhsT=v_nat[:, t, :],
                                rhs=pT[:, half, :],
                                start=(t == 0), stop=(t == NT - 1),
                            )
                        # denominator: elementwise-accumulate p on DVE
                        if tp == 0:
                            nc.vector.tensor_copy(dacc[:, :, :], pT[:, :, :])
                        else:
                            nc.vector.tensor_tensor(
                                dacc[:, :, :], dacc[:, :, :], pT[:, :, :], add
                            )

                    paux = auxpool.tile([128, SBLK], F32, tag="aux")
                    psum_den = paux
                    for half in range(2):
                        nc.tensor.matmul(
                            psum_den[0:1, :],
                            lhsT=ones_col,
                            rhs=dacc[:, half, :],
                            start=(half == 0), stop=(half == 1),
                        )
                    recip_f = rcpool.tile([1, SBLK], F32, tag="rcf")
                    nc.vector.reciprocal_approx_fast(
                        recip_f[0:1, :], psum_den[0:1, :]
                    )
                    recip = rcpool.tile([1, SBLK], F32R, tag="rc")
                    nc.vector.tensor_copy(recip[0:1, :], recip_f[0:1, :])
                    pb = paux   # reuse the bank: den row was consumed by recip
                    nc.tensor.matmul(
                        pb[:, :], lhsT=ones_row[0:1, :],
                        rhs=recip[0:1, :],
                        start=True, stop=True,
                    )
                    recip_b = rcpool.tile([128, SBLK], F32, tag="rb")
                    nc.scalar.copy(recip_b[:, :], pb[:, :])
                    nc.vector.tensor_tensor(
                        outT[:, hi, :], psum_o[:, :], recip_b[:, :], mult
                    )

                    # overlap Wo DMA + rounds with the attention loop
                    if h == 3:
                        wo_round([0, 1, 2, 3], first=True)
                        wo_fetch(4)
                    elif h == 7:
                        wo_round([4, 5, 6, 7])
                        wo_fetch(8)
                    elif h == 11:
                        wo_round([8, 9, 10, 11])
                        wo_fetch(12)
                    elif h == 13:
                        wo_round([12, 13])

                wo_round([14, 15], final=True)

    nc.compile()
    return nc


def _get_nc():
    if "nc" not in _cache:
        _cache["nc"] = _build()
    return _cache["nc"]


def _prepare_in_maps(x, Wq, bq, Wk, bk, Wv, bv, Wo, bo):
    x = np.asarray(x, dtype=np.float32)
    bq = np.asarray(bq, dtype=np.float32)
    bk = np.asarray(bk, dtype=np.float32)
    bv = np.asarray(bv, dtype=np.float32)

    Wq = np.asarray(Wq, np.float32)
    Wk = np.asarray(Wk, np.float32)
    Wv = np.asarray(Wv, np.float32)
    Wo = np.asarray(Wo, np.float32)

    # [p, h, n, m] <- Wq[n*128+p, h*128+m]  (contiguous 2KB DMA lines)
    Wq_pre = _to_f16(Wq.reshape(ND, 128, HQ, 128).transpose(1, 2, 0, 3))
    Wk_pre = _to_f16(Wk.reshape(ND, 128, DH).transpose(1, 0, 2))
    Wv_pre = _to_f16(Wv.reshape(ND, 128, DH).transpose(1, 0, 2))
    # [p, h, db, m] <- Wo[h*128+p, db*512+m]
    Wo_pre = _to_f16(Wo.reshape(HQ, 128, 4, 512).transpose(1, 0, 2, 3))
    bo_pre = _round_fp32r(np.asarray(bo, np.float32)).reshape(1, D)

    ones16 = np.ones((128, 16), np.float16)
    onesr = np.ones((1, 128), np.float32)

    # xT_pre[g]: [p, n, s] <- x[g].T[n*128+p, s]
    xT_pre = [
        _to_f16(x[g].T.reshape(ND, 128, S).transpose(1, 0, 2))
        for g in range(B)
    ]
    in_maps = []
    for c in range(N_CORES):
        g, blk = divmod(c, 4)
        s0 = blk * SBLK
        in_maps.append({
            "xall": xT_pre[g],
            "xq": np.ascontiguousarray(xT_pre[g][:, :, s0:s0 + SBLK]),
            "Wq": Wq_pre, "bq": bq, "Wk": Wk_pre, "bk": bk,
            "Wv": Wv_pre, "bv": bv, "Wo": Wo_pre, "bo": bo_pre,
            "ones16": ones16, "onesr": onesr,
        })
    return in_maps


def _assemble(results):
    out = np.empty((B, S, D), dtype=np.float32)
    for c in range(N_CORES):
        g, blk = divmod(c, 4)
        out[g, blk * SBLK:(blk + 1) * SBLK, :] = results[c]["y"]
    return out


def kernel(x, Wq, bq, Wk, bk, Wv, bv, Wo, bo):
    from concourse.bass_utils import run_bass_kernel_spmd

    in_maps = _prepare_in_maps(x, Wq, bq, Wk, bk, Wv, bv, Wo, bo)
    nc = _get_nc()
    res = run_bass_kernel_spmd(nc, in_maps, core_ids=list(range(N_CORES)))
    return _assemble(res.results)


# revision 29
# speedup vs baseline: 1.4632x; 1.0346x over previous
"""Multi-head attention block (16 query heads, shared single K/V head) on
8 Trainium2 NeuronCores — fp16 pipeline.

Reference computation (B=2, S=2048, D=2048, HQ=16, DH=128, fp32):
    q = (x @ Wq + bq)  -> [B, S, 16, 128]
    k = x @ Wk + bk    -> [B, S, 128]   (single shared K/V head)
    v = x @ Wv + bv    -> [B, S, 128]
    attn = softmax(q k^T / sqrt(128))
    out = (attn @ v) reshaped -> [B, S, D];  y = out @ Wo + bo
    (dropout is identity in eval)

Sharding: batch x sequence-block data parallel (no collectives). Core c
handles batch c//4, query rows (c%4)*512 .. +512, for ALL 16 heads; K/V
over the full sequence are recomputed per core (cheap).

Precision: the attention output is a softmax-weighted mean over ~750
effective keys, so its magnitude is ~27x smaller than v's; quantization
noise on any matmul operand passes through to the output at roughly its
per-element RMS. fp8's ~4%/element is far too coarse, so every operand
runs fp16 (~0.1%/element, full PE rate, fast-weight-load eligible) with
fp32 PSUM accumulation. Measured end-to-end error vs the fp32 reference
is 6.8e-4.

Structure: scores stay in the transposed [key, query] layout end-to-end
(softmax skips max-subtraction; scores ~N(0,1) by construction, and exp
applies a constant -3 offset that cancels in the normalization). exp runs
on ScalarE straight PSUM->SBUF; p@v contracts the key axis on the PE with
no transposes. Softmax denominators come from DVE tile-accumulation of p
plus one tiny ones-matmul per head (saving ~120k PE cycles vs per-tile
ones-matmuls); the per-head normalization uses reciprocal_approx_fast and
a PE row-broadcast. The Wo projection runs in rounds (4+4+4+2+2 heads) as
heads complete, so all but the final 2-head round overlaps the attention
loop. The Q projection is software-pipelined one head ahead so the
in-order PE queue has work while each head's qT bias-add drains on DVE.
"""

import numpy as np

B, S, D = 2, 2048, 2048
HQ, DH = 16, 128
SBLK = S // 4          # 512 query rows per core
N_CORES = 8
SCALE = 1.0 / float(np.sqrt(DH))
EXP_BIAS = -3.0        # exp(s/sqrt(DH) - 3): cancels in softmax

ND = D // 128          # 16 contraction chunks
NT = S // 128          # 16 key tiles
NQ = SBLK // 128       # 4 query row-tiles per core

_cache = {}


def _round_fp32r(a):
    """Round fp32 to fp32r (1s+8e+11m) with round-to-nearest-even-ish."""
    b = np.ascontiguousarray(a, dtype=np.float32).view(np.uint32)
    bias = np.uint32(0x7FF) + ((b >> np.uint32(12)) & np.uint32(1))
    return ((b + bias) & np.uint32(0xFFFFF000)).view(np.float32)


def _to_f16(a):
    return np.ascontiguousarray(np.asarray(a, np.float32)).astype(np.float16)


def _build():
    from concourse import bacc, mybir, tile
    from concourse.masks import make_identity

    F32 = mybir.dt.float32
    F32R = mybir.dt.float32r
    F16 = mybir.dt.float16
    Exp = mybir.ActivationFunctionType.Exp
    mult = mybir.AluOpType.mult
    add = mybir.AluOpType.add

    nc = bacc.Bacc("TRN2", target_bir_lowering=False, debug=False,
                   num_devices=N_CORES)

    # pre-rearranged on host; see _prepare_in_maps
    xall_d = nc.dram_tensor("xall", [128, ND, S], F16, kind="ExternalInput").ap()
    xq_d = nc.dram_tensor("xq", [128, ND, SBLK], F16, kind="ExternalInput").ap()
    Wq = nc.dram_tensor("Wq", [128, HQ, ND, 128], F16, kind="ExternalInput").ap()
    bq = nc.dram_tensor("bq", [D], F32, kind="ExternalInput").ap()
    Wk = nc.dram_tensor("Wk", [128, ND, DH], F16, kind="ExternalInput").ap()
    bk = nc.dram_tensor("bk", [DH], F32, kind="ExternalInput").ap()
    Wv = nc.dram_tensor("Wv", [128, ND, DH], F16, kind="ExternalInput").ap()
    bv = nc.dram_tensor("bv", [DH], F32, kind="ExternalInput").ap()
    Wo = nc.dram_tensor("Wo", [128, HQ, 4, 512], F16, kind="ExternalInput").ap()
    bo = nc.dram_tensor("bo", [1, D], F32R, kind="ExternalInput").ap()
    ones16_d = nc.dram_tensor("ones16", [128, 16], F16, kind="ExternalInput").ap()
    onesr_d = nc.dram_tensor("onesr", [1, 128], F32R, kind="ExternalInput").ap()
    y = nc.dram_tensor("y", [SBLK, D], F32, kind="ExternalOutput").ap()

    with tile.TileContext(nc) as tc, nc.allow_low_precision(
        reason="fp16 matmul pipeline; verified against fp32 reference"
    ):
        with (
            tc.tile_pool(name="const", bufs=1) as cpool,
            tc.tile_pool(name="live", bufs=1) as lpool,
            tc.tile_pool(name="ot", bufs=HQ // 2) as otpool,  # 8 head-pair outs
            tc.tile_pool(name="ya", bufs=16) as yapool,       # y accumulators
            tc.tile_pool(name="wo", bufs=20) as wopool,
            tc.tile_pool(name="yw", bufs=3) as ypool,
            tc.tile_pool(name="rc", bufs=2) as rcpool,
        ):
            # ---- constants -------------------------------------------------
            ones16 = cpool.tile([128, 16], F16)
            nc.sync.dma_start(out=ones16[:, :], in_=ones16_d[:, :])
            ones_col = ones16[:, 0:1]
            ones_row = cpool.tile([1, 128], F32R)
            nc.sync.dma_start(out=ones_row[:, :], in_=onesr_d[:, :])
            ident = cpool.tile([128, 128], F32)
            make_identity(nc, ident[:, :])
            ebias_col = cpool.tile([128, 1], F32)
            nc.gpsimd.memset(ebias_col[:, :], EXP_BIAS)

            bk_col = cpool.tile([128, 1], F32)
            nc.sync.dma_start(out=bk_col[:, :], in_=bk[:].unsqueeze(1))
            bv_col = cpool.tile([128, 1], F32)
            nc.sync.dma_start(out=bv_col[:, :], in_=bv[:].unsqueeze(1))
            bq_cols = cpool.tile([128, HQ], F32)
            nc.sync.dma_start(
                out=bq_cols[:, :], in_=bq[:].rearrange("(h p) -> p h", p=128)
            )
            bo_row = cpool.tile([1, D], F32R)
            nc.sync.dma_start(out=bo_row[:, :], in_=bo[:, :])

            xq = lpool.tile([128, ND, SBLK], F16)
            kT = lpool.tile([128, S], F16)
            v_nat = lpool.tile([128, NT, DH], F16)

            # ---- phase A: k/v projections over the full sequence -----------
            # xall is scoped here so its 64KB/partition frees before phase B.
            with (
                tc.tile_pool(name="pha", bufs=1) as apool,
                tc.tile_pool(name="pacc", bufs=1, space="PSUM") as pacc,
                tc.tile_pool(name="ptr", bufs=2, space="PSUM") as ptrp,
            ):
                wk_all = apool.tile([128, ND, DH], F16)
                nc.sync.dma_start(out=wk_all[:, :, :], in_=Wk[:, :, :])
                wv_all = apool.tile([128, ND, DH], F16)
                nc.sync.dma_start(out=wv_all[:, :, :], in_=Wv[:, :, :])
                xall = apool.tile([128, ND, S], F16)
                for c8 in range(8):
                    nc.sync.dma_start(
                        out=xall[:, c8 * 2:(c8 + 1) * 2, :],
                        in_=xall_d[:, c8 * 2:(c8 + 1) * 2, :],
                    )
                    if c8 == 0:
                        nc.sync.dma_start(out=xq[:, :, :], in_=xq_d[:, :, :])
                vT = apool.tile([128, S], F32)

                HS = S // 2
                for th in range(2):
                    tsl = slice(th * HS, (th + 1) * HS)
                    psum_k = pacc.tile([128, HS], F32, tag="pk")
                    psum_v = pacc.tile([128, HS], F32, tag="pv")
                    for d in range(ND):
                        for nb in range(HS // 512):
                            sl = slice(nb * 512, (nb + 1) * 512)
                            xsl = slice(th * HS + nb * 512,
                                        th * HS + (nb + 1) * 512)
                            nc.tensor.matmul(
                                psum_k[:, sl],
                                lhsT=wk_all[:, d, :],
                                rhs=xall[:, d, xsl],
                                start=(d == 0), stop=(d == ND - 1),
                            )
                            nc.tensor.matmul(
                                psum_v[:, sl],
                                lhsT=wv_all[:, d, :],
                                rhs=xall[:, d, xsl],
                                start=(d == 0), stop=(d == ND - 1),
                            )

                    nc.vector.tensor_scalar(
                        kT[:, tsl], psum_k[:, :], bk_col[:, :], None, add
                    )
                    nc.vector.tensor_scalar(
                        vT[:, tsl], psum_v[:, :], bv_col[:, :], None, add
                    )
                    # v into natural [key, DH] layout for p@v; th0's half is
                    # emitted after th1's matmuls so the in-order PE queue
                    # isn't head-of-line blocked waiting on vT's bias-add.
                    if th == 1:
                        for t in range(NT // 2):
                            ptr = ptrp.tile([128, 128], F32, tag="tr")
                            nc.tensor.transpose(
                                ptr[:, :], vT[:, t * 128:(t + 1) * 128],
                                ident[:, :],
                            )
                            nc.vector.tensor_copy(v_nat[:, t, :], ptr[:, :])

                for t in range(NT // 2, NT):
                    ptr = ptrp.tile([128, 128], F32, tag="tr")
                    nc.tensor.transpose(
                        ptr[:, :], vT[:, t * 128:(t + 1) * 128], ident[:, :]
                    )
                    nc.vector.tensor_copy(v_nat[:, t, :], ptr[:, :])

            # ---- phase B (attention) + phase C (Wo) interleaved ------------
            outT_pairs = []
            yacc_tiles = {}
            wo_tiles = {}

            with (
                tc.tile_pool(name="wq", bufs=3) as wqpool,
                tc.tile_pool(name="qt", bufs=2) as qtpool,
                tc.tile_pool(name="pt", bufs=3) as ptpool,
                tc.tile_pool(name="dac", bufs=2) as dacpool,
                tc.tile_pool(name="ps", bufs=2, space="PSUM") as pspool,
                tc.tile_pool(name="po", bufs=2, space="PSUM") as popool,
                tc.tile_pool(name="aux", bufs=1, space="PSUM") as auxpool,
                tc.tile_pool(name="py", bufs=1, space="PSUM") as pypool,
            ):
                # bo broadcast [1,D] -> [128,D] via PE
                bo_b = cpool.tile([128, D], F32)
                for nb in range(D // 512):
                    pbo = pypool.tile([128, 512], F32, tag="py")
                    nc.tensor.matmul(
                        pbo[:, :], lhsT=ones_row[0:1, :],
                        rhs=bo_row[:, nb * 512:(nb + 1) * 512],
                        start=True, stop=True,
                    )
                    nc.scalar.copy(bo_b[:, nb * 512:(nb + 1) * 512], pbo[:, :])

                def wo_round(heads, first=False, final=False):
                    """Accumulate the given heads into the y accumulators."""
                    for st in range(NQ):
                        for db in range(4):
                            if final:
                                # the po ring is free once head 15's output
                                # has drained; double-buffer the tail there
                                py = popool.tile([128, 512], F32, tag="po")
                            else:
                                py = pypool.tile([128, 512], F32, tag="py")
                            for j, h in enumerate(heads):
                                pp, hi = divmod(h, 2)
                                nc.tensor.matmul(
                                    py[:, :],
                                    lhsT=outT_pairs[pp][
                                        :, hi, st * 128:(st + 1) * 128],
                                    rhs=wo_tiles[(h, db)][:, :],
                                    start=(j == 0), stop=(j == len(heads) - 1),
                                )
                            if first:
                                ya = yapool.tile([128, 512], F32, tag="ya")
                                yacc_tiles[(st, db)] = ya
                                nc.vector.tensor_tensor(
                                    ya[:, :], py[:, :],
                                    bo_b[:, db * 512:(db + 1) * 512], add,
                                )
                            elif not final:
                                ya = yacc_tiles[(st, db)]
                                nc.vector.tensor_tensor(
                                    ya[:, :], py[:, :], ya[:, :], add,
                                )
                            else:
                                ya = yacc_tiles[(st, db)]
                                y_sb = ypool.tile([128, 512], F32, tag="y")
                                nc.vector.tensor_tensor(
                                    y_sb[:, :], py[:, :], ya[:, :], add,
                                )
                                nc.sync.dma_start(
                                    out=y[st * 128:(st + 1) * 128,
                                          db * 512:(db + 1) * 512],
                                    in_=y_sb[:, :],
                                )

                def wo_fetch(h4):
                    for hh in range(h4, h4 + 4):
                        for db in range(4):
                            wt = wopool.tile([128, 512], F16, tag="wo")
                            nc.sync.dma_start(
                                out=wt[:, :], in_=Wo[:, hh, db, :]
                            )
                            wo_tiles[(hh, db)] = wt

                wq_tiles = {}

                def wq_fetch(hh):
                    wq_t = wqpool.tile([128, ND, 128], F16, tag="wq")
                    nc.sync.dma_start(out=wq_t[:, :, :], in_=Wq[:, hh, :, :])
                    wq_tiles[hh] = wq_t

                for hh in range(3):
                    wq_fetch(hh)
                wo_fetch(0)

                def qproj(hh):
                    """Q projection for head hh; emitted one head ahead so
                    these matmuls fill the PE while the previous head's qT
                    bias-add drains on DVE."""
                    wq_t = wq_tiles.pop(hh)
                    pq = pspool.tile([128, 2, SBLK], F32, tag="sc")
                    for d in range(ND):
                        nc.tensor.matmul(
                            pq[:, 0, :],
                            lhsT=wq_t[:, d, :],
                            rhs=xq[:, d, :],
                            start=(d == 0), stop=(d == ND - 1),
                        )
                    qT = qtpool.tile([128, SBLK], F16, tag="qt")
                    nc.vector.tensor_scalar(
                        qT[:, :], pq[:, 0, :], bq_cols[:, hh:hh + 1], None, add
                    )
                    return qT

                qT_next = qproj(0)

                for h in range(HQ):
                    pp, hi = divmod(h, 2)
                    if h + 3 < HQ:
                        wq_fetch(h + 3)
                    qT = qT_next
                    if h + 1 < HQ:
                        qT_next = qproj(h + 1)

                    if hi == 0:
                        outT = otpool.tile([128, 2, SBLK], F16, tag="ot")
                        outT_pairs.append(outT)
                    outT = outT_pairs[pp]

                    psum_o = popool.tile([128, SBLK], F32, tag="po")
                    dacc = dacpool.tile([128, 2, SBLK], F16, tag="da")
                    for tp in range(NT // 2):
                        psc = pspool.tile([128, 2, SBLK], F32, tag="sc")
                        for half in range(2):
                            t = tp * 2 + half
                            nc.tensor.matmul(
                                psc[:, half, :],
                                lhsT=kT[:, t * 128:(t + 1) * 128],
                                rhs=qT[:, :],
                                start=True, stop=True,
                            )
                        pT = ptpool.tile([128, 2, SBLK], F16, tag="pT")
                        nc.scalar.activation(
                            pT[:, :, :], psc[:, :, :], Exp,
                            bias=ebias_col[:, :], scale=SCALE,
                        )
                        for half in range(2):
                            t = tp * 2 + half
                            nc.tensor.matmul(
                                psum_o[:, :],
                                lhsT=v_nat[:, t, :],
                                rhs=pT[:, half, :],
                                start=(t == 0), stop=(t == NT - 1),
                            )
                        # denominator: elementwise-accumulate p on DVE
                        if tp == 0:
                            nc.vector.tensor_copy(dacc[:, :, :], pT[:, :, :])
                        else:
                            nc.vector.tensor_tensor(
                                dacc[:, :, :], dacc[:, :, :], pT[:, :, :], add
                            )

                    paux = auxpool.tile([128, SBLK], F32, tag="aux")
                    psum_den = paux
                    for half in range(2):
                        nc.tensor.matmul(
                            psum_den[0:1, :],
                            lhsT=ones_col,
                            rhs=dacc[:, half, :],
                            start=(half == 0), stop=(half == 1),
                        )
                    recip_f = rcpool.tile([1, SBLK], F32, tag="rcf")
                    nc.vector.reciprocal_approx_fast(
                        recip_f[0:1, :], psum_den[0:1, :]
                    )
                    recip = rcpool.tile([1, SBLK], F32R, tag="rc")
                    nc.vector.tensor_copy(recip[0:1, :], recip_f[0:1, :])
                    pb = paux   # reuse the bank: den row was consumed by recip
                    nc.tensor.matmul(
                        pb[:, :], lhsT=ones_row[0:1, :],
                        rhs=recip[0:1, :],
                        start=True, stop=True,
                    )
                    recip_b = rcpool.tile([128, SBLK], F32, tag="rb")
                    nc.scalar.copy(recip_b[:, :], pb[:, :])
                    nc.vector.tensor_tensor(
                        outT[:, hi, :], psum_o[:, :], recip_b[:, :], mult
                    )

                    # overlap Wo DMA + rounds with the attention loop
                    if h == 5:
                        wo_round([0, 1, 2, 3], first=True)
                        wo_fetch(4)
                    elif h == 8:
                        wo_round([4, 5, 6, 7])
                        wo_fetch(8)
                    elif h == 11:
                        wo_round([8, 9, 10, 11])
                        wo_fetch(12)
                    elif h == 13:
                        wo_round([12, 13])

                wo_round([14, 15], final=True)

    nc.compile()
    return nc


def _get_nc():
    if "nc" not in _cache:
        _cache["nc"] = _build()
    return _cache["nc"]


def _prepare_in_maps(x, Wq, bq, Wk, bk, Wv, bv, Wo, bo):
    x = np.asarray(x, dtype=np.float32)
    bq = np.asarray(bq, dtype=np.float32)
    bk = np.asarray(bk, dtype=np.float32)
    bv = np.asarray(bv, dtype=np.float32)

    Wq = np.asarray(Wq, np.float32)
    Wk = np.asarray(Wk, np.float32)
    Wv = np.asarray(Wv, np.float32)
    Wo = np.asarray(Wo, np.float32)

    # [p, h, n, m] <- Wq[n*128+p, h*128+m]  (contiguous 2KB DMA lines)
    Wq_pre = _to_f16(Wq.reshape(ND, 128, HQ, 128).transpose(1, 2, 0, 3))
    Wk_pre = _to_f16(Wk.reshape(ND, 128, DH).transpose(1, 0, 2))
    Wv_pre = _to_f16(Wv.reshape(ND, 128, DH).transpose(1, 0, 2))
    # [p, h, db, m] <- Wo[h*128+p, db*512+m]
    Wo_pre = _to_f16(Wo.reshape(HQ, 128, 4, 512).transpose(1, 0, 2, 3))
    bo_pre = _round_fp32r(np.asarray(bo, np.float32)).reshape(1, D)

    ones16 = np.ones((128, 16), np.float16)
    onesr = np.ones((1, 128), np.float32)

    # xT_pre[g]: [p, n, s] <- x[g].T[n*128+p, s]
    xT_pre = [
        _to_f16(x[g].T.reshape(ND, 128, S).transpose(1, 0, 2))
        for g in range(B)
    ]
    in_maps = []
    for c in range(N_CORES):
        g, blk = divmod(c, 4)
        s0 = blk * SBLK
        in_maps.append({
            "xall": xT_pre[g],
            "xq": np.ascontiguousarray(xT_pre[g][:, :, s0:s0 + SBLK]),
            "Wq": Wq_pre, "bq": bq, "Wk": Wk_pre, "bk": bk,
            "Wv": Wv_pre, "bv": bv, "Wo": Wo_pre, "bo": bo_pre,
            "ones16": ones16, "onesr": onesr,
        })
    return in_maps


def _assemble(results):
    out = np.empty((B, S, D), dtype=np.float32)
    for c in range(N_CORES):
        g, blk = divmod(c, 4)
        out[g, blk * SBLK:(blk + 1) * SBLK, :] = results[c]["y"]
    return out


def kernel(x, Wq, bq, Wk, bk, Wv, bv, Wo, bo):
    from concourse.bass_utils import run_bass_kernel_spmd

    in_maps = _prepare_in_maps(x, Wq, bq, Wk, bk, Wv, bv, Wo, bo)
    nc = _get_nc()
    res = run_bass_kernel_spmd(nc, in_maps, core_ids=list(range(N_CORES)))
    return _assemble(res.results)
